# revision 1
# baseline (speedup 1.0000x reference)
"""Trainium2 Bass kernel for nn_AttentionBlock (GroupNorm + MHA + proj + residual).

Full inputs in, full output out. Sharding: 8 cores = 2 batches x 4 query-slices.
Each core: GroupNorm over its batch image (replicated within the batch group),
q projection for its 1024 queries, k/v projections over all 4096 keys,
per-head attention (S^T = k^T q formulation, softmax along the PSUM partition
axis via an appended ones-column in the PV matmul), output projection and
residual for its query slice. Host side only slices/rotates/concatenates.

All matmuls run in bf16 with fp32 PSUM accumulation; softmax logits stay fp32.
"""
import numpy as np

C = 512          # channels
N = 4096         # pixels (64*64)
NQ = 1024        # queries per core
H = 8            # heads
D = 64           # head dim
T = 4            # 128-channel chunks
W = NQ // 512    # query windows of 512
MT = N // 128    # key m-tiles of 128
NGROUPS = 8
EPS = 1e-5
GELEM = (C // NGROUPS) * N   # elements per norm group
MG = [3, 2] * 6 + [2]        # alternating m-tile group sizes (3+2 psum banks)

DEBUG = False                # adds intermediate-dump DRAM outputs

_COMPILED = None


def _emit(tc, io):
    import concourse.bass as bass
    from concourse import mybir, bass_isa
    from contextlib import ExitStack

    nc = tc.nc
    f32 = mybir.dt.float32
    bf16 = mybir.dt.bfloat16
    Alu = mybir.AluOpType
    Act = mybir.ActivationFunctionType

    xb, qkvw, qkvb, projw, projb, nw, nb, y = (
        io["xb"], io["qkvw"], io["qkvb"], io["projw"], io["projb"],
        io["nw"], io["nb"], io["y"])

    ctx = ExitStack()
    with ctx:
        # ---------------- pools ----------------
        # PSUM: big pool 2x(128,1536) [6 banks] shared by S-tiles, phase-3
        # accumulators and weight transposes; pv gets its own bank; bc/proj
        # share one more. 6+1+1 = 8 banks.
        left = ctx.enter_context(tc.tile_pool(name="left", bufs=1))
        psum_big = ctx.enter_context(tc.tile_pool(name="psum_big", bufs=1, space="PSUM"))
        psum_s2 = ctx.enter_context(tc.tile_pool(name="psum_s2", bufs=1, space="PSUM"))
        psum_pv = ctx.enter_context(tc.tile_pool(name="psum_pv", bufs=2, space="PSUM"))
        psum_acc = ctx.enter_context(tc.tile_pool(name="psum_acc", bufs=1, space="PSUM"))

        right_ctx = ExitStack()
        xf_pool = right_ctx.enter_context(
            tc.tile_pool(name="xf_pool", bufs=1, side="right"))
        wstg_pool = right_ctx.enter_context(
            tc.tile_pool(name="wstg_pool", bufs=4, side="right"))
        scr_pool = right_ctx.enter_context(
            tc.tile_pool(name="scr_pool", bufs=2, side="right"))

        # ---------------- persistent tiles ----------------
        xn = [left.tile([128, N], bf16, name=f"xn{t}", tag=f"xn{t}") for t in range(T)]
        ksb = [left.tile([128, N], bf16, name=f"ksb{t}", tag=f"ksb{t}") for t in range(T)]
        qsb = [left.tile([128, NQ], bf16, name=f"qsb{t}", tag=f"qsb{t}") for t in range(T)]
        wTq = [left.tile([128, 1536], bf16, name=f"wTq{t}", tag=f"wTq{t}") for t in range(T)]
        wTp = [left.tile([128, C], bf16, name=f"wTp{t}", tag=f"wTp{t}") for t in range(T)]
        vb_bc = left.tile([128, C], f32, name="vb_bc", tag="vb_bc")
        ones_row = left.tile([1, D], f32, name="ones_row", tag="ones_row")
        qb = [left.tile([128, 1], f32, name=f"qb{i}", tag=f"qb{i}") for i in range(8)]
        pb = [left.tile([128, 1], f32, name=f"pb{i}", tag=f"pb{i}") for i in range(T)]
        nwt = [left.tile([128, 1], f32, name=f"nwt{t}", tag=f"nwt{t}") for t in range(T)]
        nbt = [left.tile([128, 1], f32, name=f"nbt{t}", tag=f"nbt{t}") for t in range(T)]
        stat = [left.tile([128, 2], f32, name=f"stat{t}", tag=f"stat{t}") for t in range(T)]
        gstat = [left.tile([128, 2], f32, name=f"gstat{t}", tag=f"gstat{t}") for t in range(T)]

        # ---------------- input DMAs ----------------
        xf = [xf_pool.tile([128, N], f32, name=f"xf{t}", tag=f"xf{t}") for t in range(T)]
        for t in range(T):
            for c4 in range(4):   # split across DMA queues
                nc.sync.dma_start(
                    xf[t][:, 1024 * c4:1024 * (c4 + 1)],
                    xb[128 * t:128 * (t + 1), 1024 * c4:1024 * (c4 + 1)])
            nc.sync.dma_start(nwt[t][:, 0:1], nw[128 * t:128 * (t + 1)])
            nc.sync.dma_start(nbt[t][:, 0:1], nb[128 * t:128 * (t + 1)])
            nc.sync.dma_start(pb[t][:, 0:1], projb[128 * t:128 * (t + 1)])
        for i in range(8):
            nc.sync.dma_start(qb[i][:, 0:1], qkvb[128 * i:128 * (i + 1)])
        # v bias broadcast to 128 partitions (stride-0 partition read)
        nc.gpsimd.dma_start(
            out=vb_bc[:],
            in_=bass.AP(tensor=qkvb.tensor, offset=1024, ap=[[0, 128], [1, C]]))
        nc.vector.memset(ones_row[0:1, :], 1.0)

        # weights: natural-layout contiguous DMA, cast to bf16, transpose
        # 128x128 blocks on the PE (identity trick) into wTq/wTp.
        # identity + group-indicator matrices come in as constant inputs
        # (gpsimd ucode for iota/affine_select is unavailable here)
        ident = left.tile([128, 128], bf16, name="ident", tag="ident")
        nc.sync.dma_start(ident[:], io["cid"][:, :])
        ind = left.tile([128, 2], f32, name="ind", tag="ind")
        nc.sync.dma_start(ind[:], io["cind"][:, :])
        indT = left.tile([2, 128], f32, name="indT", tag="indT")
        nc.sync.dma_start(indT[0:2, :], io["cindT"][:, :])
        for i in range(12):   # qkv_w row-tiles
            wstg = wstg_pool.tile([128, C], f32, name="wstg", tag="wstg")
            nc.sync.dma_start(wstg[:], qkvw[128 * i:128 * (i + 1), :])
            wbf = wstg_pool.tile([128, C], bf16, name="wbf", tag="wbf")
            nc.vector.tensor_copy(wbf[:], wstg[:])
            for j in range(T):
                tp = psum_big.tile([128, 128], bf16, name="tp", tag="sbig")
                nc.tensor.transpose(tp[:], wbf[:, 128 * j:128 * (j + 1)], ident[:])
                nc.vector.tensor_copy(wTq[j][:, 128 * i:128 * (i + 1)], tp[:])
        for i in range(4):    # proj_w row-tiles
            wstg = wstg_pool.tile([128, C], f32, name="wstg", tag="wstg")
            nc.sync.dma_start(wstg[:], projw[128 * i:128 * (i + 1), :])
            wbf = wstg_pool.tile([128, C], bf16, name="wbf", tag="wbf")
            nc.vector.tensor_copy(wbf[:], wstg[:])
            for j in range(T):
                tp = psum_big.tile([128, 128], bf16, name="tp", tag="sbig")
                nc.tensor.transpose(tp[:], wbf[:, 128 * j:128 * (j + 1)], ident[:])
                nc.vector.tensor_copy(wTp[j][:, 128 * i:128 * (i + 1)], tp[:])

        # ---------------- phase 1: group stats ----------------
        for t in range(T):
            nc.vector.tensor_reduce(
                out=stat[t][:, 0:1], in_=xf[t][:], axis=mybir.AxisListType.X, op=Alu.add)
            sq_scr = scr_pool.tile([128, N], bf16, name="sq_scr", tag="sq_scr")
            nc.scalar.activation(
                sq_scr[:], xf[t][:], Act.Square, accum_out=stat[t][:, 1:2])
            # group-reduce over partitions via indicator matmuls:
            # gg[g,s] = sum_ch ind[ch,g]*stat[ch,s]; then broadcast back
            # per channel: gstat[ch,s] = sum_g indT[g,ch]*gg[g,s]
            gg_ps = psum_acc.tile([2, 2], f32, name="gg_ps", tag="acc")
            nc.tensor.matmul(gg_ps[0:2, :], ind[:, 0:2], stat[t][:, 0:2],
                             start=True, stop=True)
            gg_sb = left.tile([2, 2], f32, name=f"gg_sb{t}", tag=f"gg_sb{t}")
            nc.vector.tensor_copy(gg_sb[0:2, :], gg_ps[0:2, :])
            gb_ps = psum_acc.tile([128, 2], f32, name="gb_ps", tag="acc")
            nc.tensor.matmul(gb_ps[:, 0:2], indT[0:2, :], gg_sb[0:2, :],
                             start=True, stop=True)
            nc.vector.tensor_copy(gstat[t][:, 0:2], gb_ps[:, 0:2])
            # mean/var/rstd -> per-channel affine a,b
            mean_t = left.tile([128, 1], f32, name=f"mean{t}", tag=f"mean{t}")
            e2_t = left.tile([128, 1], f32, name=f"e2{t}", tag=f"e2{t}")
            var_t = left.tile([128, 1], f32, name=f"var{t}", tag=f"var{t}")
            std_t = left.tile([128, 1], f32, name=f"std{t}", tag=f"std{t}")
            a_t = left.tile([128, 1], f32, name=f"a{t}", tag=f"a{t}")
            b_t = left.tile([128, 1], f32, name=f"b{t}", tag=f"b{t}")
            inv = 1.0 / GELEM
            nc.vector.tensor_scalar(mean_t[:], gstat[t][:, 0:1], inv, None, Alu.mult)
            nc.vector.tensor_scalar(e2_t[:], gstat[t][:, 1:2], inv, None, Alu.mult)
            nc.vector.scalar_tensor_tensor(
                var_t[:], mean_t[:], -1.0, mean_t[:], Alu.mult, Alu.mult)
            nc.vector.scalar_tensor_tensor(
                var_t[:], e2_t[:], EPS, var_t[:], Alu.add, Alu.add)
            nc.scalar.activation(std_t[:], var_t[:], Act.Sqrt)
            nc.vector.reciprocal(a_t[:], std_t[:])
            nc.vector.tensor_tensor(a_t[:], a_t[:], nwt[t][:], Alu.mult)
            nc.vector.tensor_tensor(b_t[:], mean_t[:], a_t[:], Alu.mult)
            nc.vector.tensor_tensor(b_t[:], nbt[t][:], b_t[:], Alu.subtract)
            # phase 2: normalize + cast
            nc.vector.tensor_scalar(
                xn[t][:], xf[t][:], a_t[:, 0:1], b_t[:, 0:1], Alu.mult, Alu.add)
            if DEBUG:
                d = io["dbg_ab"]
                nc.sync.dma_start(d[128 * t:128 * t + 128, 0:1], a_t[:])
                nc.sync.dma_start(d[128 * t:128 * t + 128, 1:2], b_t[:])
                nc.sync.dma_start(d[128 * t:128 * t + 128, 2:3], stat[t][:, 0:1])
                nc.sync.dma_start(d[128 * t:128 * t + 128, 3:4], stat[t][:, 1:2])
                nc.sync.dma_start(d[128 * t:128 * t + 128, 4:5], gstat[t][:, 0:1])
                nc.sync.dma_start(d[128 * t:128 * t + 128, 5:6], gstat[t][:, 1:2])
                nc.sync.dma_start(io["dbg_xn"][128 * t:128 * t + 128, :], xn[t][:, 0:64])

        right_ctx.close()

        # ---------------- mid pools (reuse xf space) ----------------
        mid = ctx.enter_context(tc.tile_pool(name="mid", bufs=1))
        psb_pool = ctx.enter_context(tc.tile_pool(name="psb_pool", bufs=4))
        rec_pool = ctx.enter_context(tc.tile_pool(name="rec_pool", bufs=2))
        yo_pool = ctx.enter_context(tc.tile_pool(name="yo_pool", bufs=2))

        vT = mid.tile([128, MT * 520], bf16, name="vT", tag="vT")
        attn = [mid.tile([128, NQ], bf16, name=f"attn{t}", tag=f"attn{t}") for t in range(T)]
        xres = [mid.tile([128, NQ], f32, name=f"xres{t}", tag=f"xres{t}") for t in range(T)]
        for t in range(T):
            nc.sync.dma_start(xres[t][:], xb[128 * t:128 * (t + 1), 0:NQ])

        # ones columns of the augmented v^T (denominator trick)
        ones_view = vT[:].rearrange("p (m h x) -> p m h x", m=MT, x=65)[:, :, :, 64:65]
        nc.vector.memset(ones_view, 1.0)

        # ---------------- phase 3: projections ----------------
        # q: out rows 0..511 of qkv, only NQ query columns
        for i in range(T):
            for w in range(W):
                if (i * W + w) % 2 == 0:
                    qp = psum_big.tile([128, 512], f32, name="qp", tag="sbig")
                else:
                    qp = psum_s2.tile([128, 512], f32, name="qp2", tag="s2")
                for k in range(T):
                    nc.tensor.matmul(
                        qp[:], wTq[k][:, 128 * i:128 * i + 128],
                        xn[k][:, 512 * w:512 * w + 512],
                        start=(k == 0), stop=(k == T - 1))
                nc.vector.tensor_scalar(
                    qsb[i][:, 512 * w:512 * w + 512], qp[:], qb[i][:, 0:1], None, Alu.add)
        # k: out rows 512..1023, all N columns
        for i in range(T):
            for w in range(N // 512):
                if (i * 8 + w) % 2 == 0:
                    kp = psum_big.tile([128, 512], f32, name="kp", tag="sbig")
                else:
                    kp = psum_s2.tile([128, 512], f32, name="kp2", tag="s2")
                for k in range(T):
                    nc.tensor.matmul(
                        kp[:], wTq[k][:, 512 + 128 * i:512 + 128 * i + 128],
                        xn[k][:, 512 * w:512 * w + 512],
                        start=(k == 0), stop=(k == T - 1))
                nc.vector.tensor_scalar(
                    ksb[i][:, 512 * w:512 * w + 512], kp[:], qb[4 + i][:, 0:1], None, Alu.add)
        # vT: (m, 512) per m-tile, strided into the 65-column augmented layout
        for mt in range(MT):
            if mt % 2 == 0:
                vp = psum_big.tile([128, 512], f32, name="vp", tag="sbig")
            else:
                vp = psum_s2.tile([128, 512], f32, name="vp2", tag="s2")
            for k in range(T):
                nc.tensor.matmul(
                    vp[:], xn[k][:, 128 * mt:128 * mt + 128],
                    wTq[k][:, 1024:1536],
                    start=(k == 0), stop=(k == T - 1))
            dst = vT[:, 520 * mt:520 * mt + 520].rearrange(
                "p (h x) -> p h x", x=65)[:, :, 0:64]
            src = vp[:].rearrange("p (h x) -> p h x", x=64)
            vbv = vb_bc[:].rearrange("p (h x) -> p h x", x=64)
            nc.vector.tensor_tensor(dst, src, vbv, Alu.add)
            if DEBUG and mt == 0:
                nc.sync.dma_start(io["dbg_vt"][:], vT[:, 0:520])
                nc.sync.dma_start(io["dbg_k"][0:128, :], ksb[0][:, 0:64])
                nc.sync.dma_start(io["dbg_q"][0:128, :], qsb[0][:, 0:64])

        # ---------------- phase 4: attention ----------------
        for w in range(W):
            for h in range(H):
                kt, pr = h // 2, 64 * (h % 2)
                pv = psum_pv.tile([128, 512], f32, name="pv", tag="pv")
                mt = 0
                for gs in MG:
                    if gs == 3:
                        sp = psum_big.tile([128, 1536], f32, name="sp", tag="sbig")
                    else:
                        sp = psum_s2.tile([128, 1024], f32, name="sp2", tag="s2")
                    for j in range(gs):
                        nc.tensor.matmul(
                            sp[:, 512 * j:512 * j + 512],
                            ksb[kt][pr:pr + 64, 128 * (mt + j):128 * (mt + j) + 128],
                            qsb[kt][pr:pr + 64, 512 * w:512 * w + 512],
                            start=True, stop=True)
                    ps = psb_pool.tile([128, 1536], bf16, name="ps", tag="ps")
                    nc.scalar.activation(
                        ps[:, 0:512 * gs], sp[:, 0:512 * gs], Act.Exp, scale=0.125)
                    for j in range(gs):
                        m = mt + j
                        nc.tensor.matmul(
                            pv[0:65, :],
                            vT[:, 520 * m + 65 * h:520 * m + 65 * h + 65],
                            ps[:, 512 * j:512 * j + 512],
                            start=(m == 0), stop=(m == MT - 1))
                    mt += gs
                # NOTE: reciprocal_approx_* mis-handles nonzero partition
                # offsets on HW (reads partition 0), so stage the denominator
                # row at partition 0 first
                dnm = rec_pool.tile([1, 512], f32, name="dnm", tag="dnm")
                nc.vector.tensor_copy(dnm[0:1, :], pv[64:65, :])
                if DEBUG and w == 0:
                    dd2 = rec_pool.tile([1, 512], f32, name="dd2", tag="dd2")
                    nc.vector.tensor_copy(dd2[0:1, :], pv[0:1, :])
                    nc.sync.dma_start(io["dbg_den"][h:h + 1, :], dnm[0:1, :])
                    nc.sync.dma_start(io["dbg_pv"][h:h + 1, :], dd2[0:1, :])
                rec = rec_pool.tile([1, 512], f32, name="rec", tag="rec")
                rscr = rec_pool.tile([1, 512], f32, name="rscr", tag="rscr")
                nc.vector.reciprocal_approx_accurate(
                    rec[0:1, :], dnm[0:1, :], rscr[0:1, :])
                bc = psum_acc.tile([128, 512], f32, name="bc", tag="acc")
                nc.tensor.matmul(
                    bc[0:64, :], ones_row[0:1, 0:D],
                    rec[0:1, :], start=True, stop=True)
                bcs = rec_pool.tile([64, 512], f32, name="bcs", tag="bcs")
                nc.vector.tensor_copy(bcs[0:64, :], bc[0:64, :])
                nc.vector.tensor_tensor(
                    attn[kt][pr:pr + 64, 512 * w:512 * w + 512],
                    pv[0:64, :], bcs[0:64, :], Alu.mult)

            # ---------------- phase 5: proj + residual for this window ----
            for i in range(T):
                py = psum_acc.tile([128, 512], f32, name="py", tag="acc")
                # shares the 1-bank acc pool with bc; proj overlaps attention
                # of the next window only through this slot
                for k in range(T):
                    nc.tensor.matmul(
                        py[:], wTp[k][:, 128 * i:128 * i + 128],
                        attn[k][:, 512 * w:512 * w + 512],
                        start=(k == 0), stop=(k == T - 1))
                yo = yo_pool.tile([128, 512], f32, name="yo", tag="yo")
                nc.vector.scalar_tensor_tensor(
                    yo[:], py[:], pb[i][:, 0:1], xres[i][:, 512 * w:512 * w + 512],
                    Alu.add, Alu.add)
                nc.sync.dma_start(y[128 * i:128 * i + 128, 512 * w:512 * w + 512], yo[:])


def _build():
    import concourse.tile as tile
    from concourse import bacc, mybir

    nc = bacc.Bacc("TRN2", target_bir_lowering=False, debug=False)
    f32 = mybir.dt.float32
    io = {
        "xb": nc.dram_tensor("xb", [C, N], f32, kind="ExternalInput").ap(),
        "qkvw": nc.dram_tensor("qkvw", [3 * C, C], f32, kind="ExternalInput").ap(),
        "qkvb": nc.dram_tensor("qkvb", [3 * C], f32, kind="ExternalInput").ap(),
        "projw": nc.dram_tensor("projw", [C, C], f32, kind="ExternalInput").ap(),
        "projb": nc.dram_tensor("projb", [C], f32, kind="ExternalInput").ap(),
        "nw": nc.dram_tensor("nw", [C], f32, kind="ExternalInput").ap(),
        "nb": nc.dram_tensor("nb", [C], f32, kind="ExternalInput").ap(),
        "cid": nc.dram_tensor("cid", [128, 128], mybir.dt.bfloat16,
                              kind="ExternalInput").ap(),
        "cind": nc.dram_tensor("cind", [128, 2], f32, kind="ExternalInput").ap(),
        "cindT": nc.dram_tensor("cindT", [2, 128], f32, kind="ExternalInput").ap(),
        "y": nc.dram_tensor("y", [C, NQ], f32, kind="ExternalOutput").ap(),
    }
    if DEBUG:
        bf16 = mybir.dt.bfloat16
        io["dbg_ab"] = nc.dram_tensor("dbg_ab", [C, 8], f32, kind="ExternalOutput").ap()
        io["dbg_xn"] = nc.dram_tensor("dbg_xn", [C, 64], bf16, kind="ExternalOutput").ap()
        io["dbg_vt"] = nc.dram_tensor("dbg_vt", [128, 520], bf16, kind="ExternalOutput").ap()
        io["dbg_k"] = nc.dram_tensor("dbg_k", [C, 64], bf16, kind="ExternalOutput").ap()
        io["dbg_q"] = nc.dram_tensor("dbg_q", [C, 64], bf16, kind="ExternalOutput").ap()
        io["dbg_den"] = nc.dram_tensor("dbg_den", [8, 512], f32, kind="ExternalOutput").ap()
        io["dbg_pv"] = nc.dram_tensor("dbg_pv", [8, 512], f32, kind="ExternalOutput").ap()
    with tile.TileContext(nc) as tc:
        _emit(tc, io)
    nc.compile()
    return nc


def get_compiled():
    global _COMPILED
    if _COMPILED is None:
        _COMPILED = _build()
    return _COMPILED


def make_in_maps(x, norm_w, norm_b, qkv_w, qkv_b, proj_w, proj_b):
    import ml_dtypes

    xf = np.ascontiguousarray(np.asarray(x, np.float32)).reshape(2, C, N)
    ind = np.zeros((128, 2), np.float32)
    ind[0:64, 0] = 1.0
    ind[64:128, 1] = 1.0
    shared = {
        "cid": np.eye(128, dtype=ml_dtypes.bfloat16),
        "cind": ind,
        "cindT": np.ascontiguousarray(ind.T),
        "qkvw": np.ascontiguousarray(np.asarray(qkv_w, np.float32)),
        "qkvb": np.ascontiguousarray(np.asarray(qkv_b, np.float32)),
        "projw": np.ascontiguousarray(np.asarray(proj_w, np.float32)),
        "projb": np.ascontiguousarray(np.asarray(proj_b, np.float32)),
        "nw": np.ascontiguousarray(np.asarray(norm_w, np.float32)),
        "nb": np.ascontiguousarray(np.asarray(norm_b, np.float32)),
    }
    in_maps = []
    for core in range(8):
        bi, qs = core // 4, core % 4
        # rotate so this core's queries are always columns [0:NQ)
        xroll = np.concatenate(
            [xf[bi][:, qs * NQ:], xf[bi][:, :qs * NQ]], axis=1)
        m = dict(shared)
        m["xb"] = np.ascontiguousarray(xroll)
        in_maps.append(m)
    return in_maps


def assemble(results, x):
    y = np.zeros((2, C, N), np.float32)
    for core in range(8):
        bi, qs = core // 4, core % 4
        y[bi][:, qs * NQ:(qs + 1) * NQ] = results[core]["y"]
    return y.reshape(x.shape)


def kernel(x, norm_w, norm_b, qkv_w, qkv_b, proj_w, proj_b, **_ignored):
    from concourse import bass_utils

    nc = get_compiled()
    in_maps = make_in_maps(x, norm_w, norm_b, qkv_w, qkv_b, proj_w, proj_b)
    res = bass_utils.run_bass_kernel_spmd(nc, in_maps, core_ids=list(range(8)))
    return assemble(res.results, np.asarray(x))



# revision 3
# speedup vs baseline: 1.2704x; 1.2704x over previous
"""Trainium2 Bass kernel for nn_AttentionBlock (GroupNorm + MHA + proj + residual).

Full inputs in, full output out. Sharding: 8 cores = 2 batches x 4 query-slices.
Each core: GroupNorm over its batch image (replicated within the batch group),
q projection for its 1024 queries, k/v projections over all 4096 keys,
per-head attention (S^T = k^T q formulation, softmax along the PSUM partition
axis via an appended ones-column in the PV matmul), output projection and
residual for its query slice. Host side only slices/rotates/concatenates.

v2: phase 4 is software-pipelined per head-PAIR: the even head's S groups live
in a 3-bank PSUM pool A, the odd head's in pool B (plus 2 PV banks = 8).
Softmax exp runs as one N=1536 activation per group so ScalarE (the kernel's
throughput floor: ~2 exps of 16K elems per query-window per head) streams with
no gaps; PV matmuls are emitted one period behind S so the tensor engine FIFO
never stalls behind an exp dependency.

All matmuls run in bf16 with fp32 PSUM accumulation; softmax logits stay fp32.
"""
import numpy as np

C = 512          # channels
N = 4096         # pixels (64*64)
NQ = 1024        # queries per core
H = 8            # heads
D = 64           # head dim
T = 4            # 128-channel chunks
W = NQ // 512    # query windows of 512
MT = N // 128    # key m-tiles of 128
NGROUPS = 8
EPS = 1e-5
GELEM = (C // NGROUPS) * N   # elements per norm group
NGRP = 11                    # m-groups per head stream: [3]*10 + [2]

_COMPILED = None


def _emit(tc, io):
    import concourse.bass as bass
    from concourse import mybir
    from contextlib import ExitStack

    nc = tc.nc
    f32 = mybir.dt.float32
    bf16 = mybir.dt.bfloat16
    Alu = mybir.AluOpType
    Act = mybir.ActivationFunctionType

    xb, qkvw, qkvb, projw, projb, nw, nb, y = (
        io["xb"], io["qkvw"], io["qkvb"], io["projw"], io["projb"],
        io["nw"], io["nb"], io["y"])

    ctx = ExitStack()
    with ctx:
        # ---------------- pools ----------------
        # PSUM: pool A (3 banks) = even-head S stream, pool B (3 banks) =
        # odd-head S stream, pv pool 2x1 bank. 3+3+2 = 8 banks. Phase 1/3/5
        # transposes/projection chains borrow A/B between attention uses.
        left = ctx.enter_context(tc.tile_pool(name="left", bufs=1))
        psum_a = ctx.enter_context(tc.tile_pool(name="psum_a", bufs=1, space="PSUM"))
        psum_b = ctx.enter_context(tc.tile_pool(name="psum_b", bufs=1, space="PSUM"))
        psum_pv = ctx.enter_context(tc.tile_pool(name="psum_pv", bufs=2, space="PSUM"))
        pool_ab = [psum_a, psum_b]

        right_ctx = ExitStack()
        xf_pool = right_ctx.enter_context(
            tc.tile_pool(name="xf_pool", bufs=1, side="right"))
        wstg_pool = right_ctx.enter_context(
            tc.tile_pool(name="wstg_pool", bufs=4, side="right"))
        scr_pool = right_ctx.enter_context(
            tc.tile_pool(name="scr_pool", bufs=2, side="right"))

        # ---------------- persistent tiles ----------------
        xn = [left.tile([128, N], bf16, name=f"xn{t}", tag=f"xn{t}") for t in range(T)]
        ksb = [left.tile([128, N], bf16, name=f"ksb{t}", tag=f"ksb{t}") for t in range(T)]
        qsb = [left.tile([128, NQ], bf16, name=f"qsb{t}", tag=f"qsb{t}") for t in range(T)]
        wTq = [left.tile([128, 1536], bf16, name=f"wTq{t}", tag=f"wTq{t}") for t in range(T)]
        wTp = [left.tile([128, C], bf16, name=f"wTp{t}", tag=f"wTp{t}") for t in range(T)]
        vb_bc = left.tile([128, C], f32, name="vb_bc", tag="vb_bc")
        ones_row = left.tile([1, D], f32, name="ones_row", tag="ones_row")
        qb = [left.tile([128, 1], f32, name=f"qb{i}", tag=f"qb{i}") for i in range(8)]
        pb = [left.tile([128, 1], f32, name=f"pb{i}", tag=f"pb{i}") for i in range(T)]
        nwt = [left.tile([128, 1], f32, name=f"nwt{t}", tag=f"nwt{t}") for t in range(T)]
        nbt = [left.tile([128, 1], f32, name=f"nbt{t}", tag=f"nbt{t}") for t in range(T)]
        stat = [left.tile([128, 2], f32, name=f"stat{t}", tag=f"stat{t}") for t in range(T)]
        gstat = [left.tile([128, 2], f32, name=f"gstat{t}", tag=f"gstat{t}") for t in range(T)]

        # ---------------- input DMAs ----------------
        xf = [xf_pool.tile([128, N], f32, name=f"xf{t}", tag=f"xf{t}") for t in range(T)]
        for t in range(T):
            for c4 in range(4):   # split across DMA queues
                nc.sync.dma_start(
                    xf[t][:, 1024 * c4:1024 * (c4 + 1)],
                    xb[128 * t:128 * (t + 1), 1024 * c4:1024 * (c4 + 1)])
            nc.sync.dma_start(nwt[t][:, 0:1], nw[128 * t:128 * (t + 1)])
            nc.sync.dma_start(nbt[t][:, 0:1], nb[128 * t:128 * (t + 1)])
            nc.sync.dma_start(pb[t][:, 0:1], projb[128 * t:128 * (t + 1)])
        for i in range(8):
            nc.sync.dma_start(qb[i][:, 0:1], qkvb[128 * i:128 * (i + 1)])
        # v bias broadcast to 128 partitions (stride-0 partition read)
        nc.gpsimd.dma_start(
            out=vb_bc[:],
            in_=bass.AP(tensor=qkvb.tensor, offset=1024, ap=[[0, 128], [1, C]]))
        nc.vector.memset(ones_row[0:1, :], 1.0)

        # weights: natural-layout contiguous DMA, cast to bf16, transpose
        # 128x128 blocks on the PE (identity trick) into wTq/wTp.
        ident = left.tile([128, 128], bf16, name="ident", tag="ident")
        nc.sync.dma_start(ident[:], io["cid"][:, :])
        ind = left.tile([128, 2], f32, name="ind", tag="ind")
        nc.sync.dma_start(ind[:], io["cind"][:, :])
        indT = left.tile([2, 128], f32, name="indT", tag="indT")
        nc.sync.dma_start(indT[0:2, :], io["cindT"][:, :])
        for i in range(12):   # qkv_w row-tiles
            wstg = wstg_pool.tile([128, C], f32, name="wstg", tag="wstg")
            nc.sync.dma_start(wstg[:], qkvw[128 * i:128 * (i + 1), :])
            wbf = wstg_pool.tile([128, C], bf16, name="wbf", tag="wbf")
            nc.vector.tensor_copy(wbf[:], wstg[:])
            for j in range(T):
                tp = pool_ab[i % 2].tile([128, 128], bf16, name="tp", tag="sA" if i % 2 == 0 else "sB")
                nc.tensor.transpose(tp[:], wbf[:, 128 * j:128 * (j + 1)], ident[:])
                nc.vector.tensor_copy(wTq[j][:, 128 * i:128 * (i + 1)], tp[:])
        for i in range(4):    # proj_w row-tiles
            wstg = wstg_pool.tile([128, C], f32, name="wstg", tag="wstg")
            nc.sync.dma_start(wstg[:], projw[128 * i:128 * (i + 1), :])
            wbf = wstg_pool.tile([128, C], bf16, name="wbf", tag="wbf")
            nc.vector.tensor_copy(wbf[:], wstg[:])
            for j in range(T):
                tp = pool_ab[i % 2].tile([128, 128], bf16, name="tp", tag="sA" if i % 2 == 0 else "sB")
                nc.tensor.transpose(tp[:], wbf[:, 128 * j:128 * (j + 1)], ident[:])
                nc.vector.tensor_copy(wTp[j][:, 128 * i:128 * (i + 1)], tp[:])

        # ---------------- phase 1: group stats ----------------
        for t in range(T):
            nc.vector.tensor_reduce(
                out=stat[t][:, 0:1], in_=xf[t][:], axis=mybir.AxisListType.X, op=Alu.add)
            sq_scr = scr_pool.tile([128, N], bf16, name="sq_scr", tag="sq_scr")
            nc.scalar.activation(
                sq_scr[:], xf[t][:], Act.Square, accum_out=stat[t][:, 1:2])
            # group-reduce over partitions via indicator matmuls:
            # gg[g,s] = sum_ch ind[ch,g]*stat[ch,s]; then broadcast back
            gg_ps = psum_a.tile([2, 2], f32, name="gg_ps", tag="sA")
            nc.tensor.matmul(gg_ps[0:2, :], ind[:, 0:2], stat[t][:, 0:2],
                             start=True, stop=True)
            gg_sb = left.tile([2, 2], f32, name=f"gg_sb{t}", tag=f"gg_sb{t}")
            nc.vector.tensor_copy(gg_sb[0:2, :], gg_ps[0:2, :])
            gb_ps = psum_b.tile([128, 2], f32, name="gb_ps", tag="sB")
            nc.tensor.matmul(gb_ps[:, 0:2], indT[0:2, :], gg_sb[0:2, :],
                             start=True, stop=True)
            nc.vector.tensor_copy(gstat[t][:, 0:2], gb_ps[:, 0:2])
            # mean/var/rstd -> per-channel affine a,b
            mean_t = left.tile([128, 1], f32, name=f"mean{t}", tag=f"mean{t}")
            e2_t = left.tile([128, 1], f32, name=f"e2{t}", tag=f"e2{t}")
            var_t = left.tile([128, 1], f32, name=f"var{t}", tag=f"var{t}")
            std_t = left.tile([128, 1], f32, name=f"std{t}", tag=f"std{t}")
            a_t = left.tile([128, 1], f32, name=f"a{t}", tag=f"a{t}")
            b_t = left.tile([128, 1], f32, name=f"b{t}", tag=f"b{t}")
            inv = 1.0 / GELEM
            nc.vector.tensor_scalar(mean_t[:], gstat[t][:, 0:1], inv, None, Alu.mult)
            nc.vector.tensor_scalar(e2_t[:], gstat[t][:, 1:2], inv, None, Alu.mult)
            nc.vector.scalar_tensor_tensor(
                var_t[:], mean_t[:], -1.0, mean_t[:], Alu.mult, Alu.mult)
            nc.vector.scalar_tensor_tensor(
                var_t[:], e2_t[:], EPS, var_t[:], Alu.add, Alu.add)
            nc.scalar.activation(std_t[:], var_t[:], Act.Sqrt)
            nc.vector.reciprocal(a_t[:], std_t[:])
            nc.vector.tensor_tensor(a_t[:], a_t[:], nwt[t][:], Alu.mult)
            nc.vector.tensor_tensor(b_t[:], mean_t[:], a_t[:], Alu.mult)
            nc.vector.tensor_tensor(b_t[:], nbt[t][:], b_t[:], Alu.subtract)
            # phase 2: normalize + cast
            nc.vector.tensor_scalar(
                xn[t][:], xf[t][:], a_t[:, 0:1], b_t[:, 0:1], Alu.mult, Alu.add)

        right_ctx.close()

        # ---------------- mid pools (reuse xf space) ----------------
        mid = ctx.enter_context(tc.tile_pool(name="mid", bufs=1))
        psb_pool = ctx.enter_context(tc.tile_pool(name="psb_pool", bufs=4))
        rec_pool = ctx.enter_context(tc.tile_pool(name="rec_pool", bufs=2))
        yo_pool = ctx.enter_context(tc.tile_pool(name="yo_pool", bufs=2))

        vT = mid.tile([128, MT * 520], bf16, name="vT", tag="vT")
        attn = [mid.tile([128, NQ], bf16, name=f"attn{t}", tag=f"attn{t}") for t in range(T)]
        xres = [mid.tile([128, NQ], f32, name=f"xres{t}", tag=f"xres{t}") for t in range(T)]
        for t in range(T):
            nc.sync.dma_start(xres[t][:], xb[128 * t:128 * (t + 1), 0:NQ])

        # ones columns of the augmented v^T (denominator trick)
        ones_view = vT[:].rearrange("p (m h x) -> p m h x", m=MT, x=65)[:, :, :, 64:65]
        nc.vector.memset(ones_view, 1.0)

        # ---------------- phase 3: projections ----------------
        # q: out rows 0..511 of qkv, only NQ query columns
        for i in range(T):
            for w in range(W):
                qp = pool_ab[(i * W + w) % 2].tile([128, 512], f32, name="qp", tag="sA" if (i * W + w) % 2 == 0 else "sB")
                for k in range(T):
                    nc.tensor.matmul(
                        qp[:], wTq[k][:, 128 * i:128 * i + 128],
                        xn[k][:, 512 * w:512 * w + 512],
                        start=(k == 0), stop=(k == T - 1))
                nc.vector.tensor_scalar(
                    qsb[i][:, 512 * w:512 * w + 512], qp[:], qb[i][:, 0:1], None, Alu.add)
        # k: out rows 512..1023, all N columns
        for i in range(T):
            for w in range(N // 512):
                kp = pool_ab[(i * 8 + w) % 2].tile([128, 512], f32, name="kp", tag="sA" if (i * 8 + w) % 2 == 0 else "sB")
                for k in range(T):
                    nc.tensor.matmul(
                        kp[:], wTq[k][:, 512 + 128 * i:512 + 128 * i + 128],
                        xn[k][:, 512 * w:512 * w + 512],
                        start=(k == 0), stop=(k == T - 1))
                nc.vector.tensor_scalar(
                    ksb[i][:, 512 * w:512 * w + 512], kp[:], qb[4 + i][:, 0:1], None, Alu.add)
        # vT: (m, 512) per m-tile, strided into the 65-column augmented layout
        for mt in range(MT):
            vp = pool_ab[mt % 2].tile([128, 512], f32, name="vp", tag="sA" if mt % 2 == 0 else "sB")
            for k in range(T):
                nc.tensor.matmul(
                    vp[:], xn[k][:, 128 * mt:128 * mt + 128],
                    wTq[k][:, 1024:1536],
                    start=(k == 0), stop=(k == T - 1))
            dst = vT[:, 520 * mt:520 * mt + 520].rearrange(
                "p (h x) -> p h x", x=65)[:, :, 0:64]
            src = vp[:].rearrange("p (h x) -> p h x", x=64)
            vbv = vb_bc[:].rearrange("p (h x) -> p h x", x=64)
            nc.vector.tensor_tensor(dst, src, vbv, Alu.add)

        # ---------------- phase 4: attention (pipelined pair streams) -----
        # Per (window, head-pair): 11 periods; period r computes both heads'
        # S group r (3 m-tiles; last group 2) + exp, and emits PV for group
        # r-1 so the PE FIFO never waits on the freshest exp.
        def gsize(r):
            return 3 if r < NGRP - 1 else MT - 3 * (NGRP - 1)

        for w in range(W):
            for p in range(4):
                pvs = [psum_pv.tile([128, 512], f32, name=f"pv{hh}", tag="pv")
                       for hh in range(2)]
                ps_t = [[None] * NGRP, [None] * NGRP]

                def emit_pv(r, p=p, pvs=pvs, ps_t=ps_t):
                    gs = gsize(r)
                    for hh in range(2):
                        h = 2 * p + hh
                        pst = ps_t[hh][r]
                        for j in range(gs):
                            m = 3 * r + j
                            nc.tensor.matmul(
                                pvs[hh][0:65, :],
                                vT[:, 520 * m + 65 * h:520 * m + 65 * h + 65],
                                pst[:, 512 * j:512 * j + 512],
                                start=(m == 0), stop=(m == MT - 1))

                for r in range(NGRP):
                    gs = gsize(r)
                    sp = [pool_ab[hh].tile([128, 512 * gs], f32,
                                           name=f"sp{hh}", tag="sA" if hh == 0 else "sB")
                          for hh in range(2)]
                    # S matmuls: h0 block first (pool A freed first), then PV
                    # of previous period between the two heads' S blocks.
                    for hh in range(2):
                        pr = 64 * hh
                        for j in range(gs):
                            m = 3 * r + j
                            nc.tensor.matmul(
                                sp[hh][:, 512 * j:512 * j + 512],
                                ksb[p][pr:pr + 64, 128 * m:128 * m + 128],
                                qsb[p][pr:pr + 64, 512 * w:512 * w + 512],
                                start=True, stop=True)
                        pst = psb_pool.tile([128, 1536], bf16, name="ps", tag="ps")
                        nc.scalar.activation(
                            pst[:, 0:512 * gs], sp[hh][:, 0:512 * gs],
                            Act.Exp, scale=0.125)
                        ps_t[hh][r] = pst
                        if hh == 0 and r > 0:
                            emit_pv(r - 1)
                emit_pv(NGRP - 1)

                # softmax denominators + normalize (both heads)
                for hh in range(2):
                    h = 2 * p + hh
                    kt, prr = h // 2, 64 * (h % 2)
                    # reciprocal_approx mis-handles nonzero partition offsets
                    # on HW: stage the denominator row at partition 0 first
                    dnm = rec_pool.tile([1, 512], f32, name="dnm", tag="dnm")
                    nc.vector.tensor_copy(dnm[0:1, :], pvs[hh][64:65, :])
                    rec = rec_pool.tile([1, 512], f32, name="rec", tag="rec")
                    rscr = rec_pool.tile([1, 512], f32, name="rscr", tag="rscr")
                    nc.vector.reciprocal_approx_accurate(
                        rec[0:1, :], dnm[0:1, :], rscr[0:1, :])
                    bc = pool_ab[hh].tile([64, 512], f32, name="bc", tag="sA" if hh == 0 else "sB")
                    nc.tensor.matmul(
                        bc[0:64, :], ones_row[0:1, 0:D],
                        rec[0:1, :], start=True, stop=True)
                    bcs = rec_pool.tile([64, 512], f32, name="bcs", tag="bcs")
                    nc.vector.tensor_copy(bcs[0:64, :], bc[0:64, :])
                    nc.vector.tensor_tensor(
                        attn[kt][prr:prr + 64, 512 * w:512 * w + 512],
                        pvs[hh][0:64, :], bcs[0:64, :], Alu.mult)

            # ---------------- phase 5: proj + residual for this window ----
            for i in range(T):
                py = pool_ab[i % 2].tile([128, 512], f32, name="py", tag="sA" if i % 2 == 0 else "sB")
                for k in range(T):
                    nc.tensor.matmul(
                        py[:], wTp[k][:, 128 * i:128 * i + 128],
                        attn[k][:, 512 * w:512 * w + 512],
                        start=(k == 0), stop=(k == T - 1))
                yo = yo_pool.tile([128, 512], f32, name="yo", tag="yo")
                nc.vector.scalar_tensor_tensor(
                    yo[:], py[:], pb[i][:, 0:1], xres[i][:, 512 * w:512 * w + 512],
                    Alu.add, Alu.add)
                nc.sync.dma_start(y[128 * i:128 * i + 128, 512 * w:512 * w + 512], yo[:])


def _build():
    import concourse.tile as tile
    from concourse import bacc, mybir

    nc = bacc.Bacc("TRN2", target_bir_lowering=False, debug=False)
    f32 = mybir.dt.float32
    io = {
        "xb": nc.dram_tensor("xb", [C, N], f32, kind="ExternalInput").ap(),
        "qkvw": nc.dram_tensor("qkvw", [3 * C, C], f32, kind="ExternalInput").ap(),
        "qkvb": nc.dram_tensor("qkvb", [3 * C], f32, kind="ExternalInput").ap(),
        "projw": nc.dram_tensor("projw", [C, C], f32, kind="ExternalInput").ap(),
        "projb": nc.dram_tensor("projb", [C], f32, kind="ExternalInput").ap(),
        "nw": nc.dram_tensor("nw", [C], f32, kind="ExternalInput").ap(),
        "nb": nc.dram_tensor("nb", [C], f32, kind="ExternalInput").ap(),
        "cid": nc.dram_tensor("cid", [128, 128], mybir.dt.bfloat16,
                              kind="ExternalInput").ap(),
        "cind": nc.dram_tensor("cind", [128, 2], f32, kind="ExternalInput").ap(),
        "cindT": nc.dram_tensor("cindT", [2, 128], f32, kind="ExternalInput").ap(),
        "y": nc.dram_tensor("y", [C, NQ], f32, kind="ExternalOutput").ap(),
    }
    with tile.TileContext(nc) as tc:
        _emit(tc, io)
    nc.compile()
    return nc


def get_compiled():
    global _COMPILED
    if _COMPILED is None:
        _COMPILED = _build()
    return _COMPILED


def make_in_maps(x, norm_w, norm_b, qkv_w, qkv_b, proj_w, proj_b):
    import ml_dtypes

    xf = np.ascontiguousarray(np.asarray(x, np.float32)).reshape(2, C, N)
    ind = np.zeros((128, 2), np.float32)
    ind[0:64, 0] = 1.0
    ind[64:128, 1] = 1.0
    shared = {
        "cid": np.eye(128, dtype=ml_dtypes.bfloat16),
        "cind": ind,
        "cindT": np.ascontiguousarray(ind.T),
        "qkvw": np.ascontiguousarray(np.asarray(qkv_w, np.float32)),
        "qkvb": np.ascontiguousarray(np.asarray(qkv_b, np.float32)),
        "projw": np.ascontiguousarray(np.asarray(proj_w, np.float32)),
        "projb": np.ascontiguousarray(np.asarray(proj_b, np.float32)),
        "nw": np.ascontiguousarray(np.asarray(norm_w, np.float32)),
        "nb": np.ascontiguousarray(np.asarray(norm_b, np.float32)),
    }
    in_maps = []
    for core in range(8):
        bi, qs = core // 4, core % 4
        # rotate so this core's queries are always columns [0:NQ)
        xroll = np.concatenate(
            [xf[bi][:, qs * NQ:], xf[bi][:, :qs * NQ]], axis=1)
        m = dict(shared)
        m["xb"] = np.ascontiguousarray(xroll)
        in_maps.append(m)
    return in_maps


def assemble(results, x):
    y = np.zeros((2, C, N), np.float32)
    for core in range(8):
        bi, qs = core // 4, core % 4
        y[bi][:, qs * NQ:(qs + 1) * NQ] = results[core]["y"]
    return y.reshape(x.shape)


def kernel(x, norm_w, norm_b, qkv_w, qkv_b, proj_w, proj_b, **_ignored):
    from concourse import bass_utils

    nc = get_compiled()
    in_maps = make_in_maps(x, norm_w, norm_b, qkv_w, qkv_b, proj_w, proj_b)
    res = bass_utils.run_bass_kernel_spmd(nc, in_maps, core_ids=list(range(8)))
    return assemble(res.results, np.asarray(x))


# revision 6
# speedup vs baseline: 1.5069x; 1.1862x over previous
"""Trainium2 Bass kernel for nn_AttentionBlock (GroupNorm + MHA + proj + residual).

Full inputs in, full output out. Sharding: 8 cores = 2 batches x 4 query-slices.
Each core: GroupNorm over its batch image (replicated within the batch group),
q projection for its 1024 queries, k/v projections over all 4096 keys,
per-head attention (S^T = k^T q formulation, softmax along the PSUM partition
axis via an appended ones-column in the PV matmul), output projection and
residual for its query slice. Host side only slices/rotates/concatenates.

v2: phase 4 is software-pipelined per head-PAIR: the even head's S groups live
in a 3-bank PSUM pool A, the odd head's in pool B (plus 2 PV banks = 8).
Softmax exp runs as one N=1536 activation per group so ScalarE (the kernel's
throughput floor: ~2 exps of 16K elems per query-window per head) streams with
no gaps; PV matmuls are emitted one period behind S so the tensor engine FIFO
never stalls behind an exp dependency.

All matmuls run in bf16 with fp32 PSUM accumulation; softmax logits stay fp32.
"""
import numpy as np

C = 512          # channels
N = 4096         # pixels (64*64)
NQ = 1024        # queries per core
H = 8            # heads
D = 64           # head dim
T = 4            # 128-channel chunks
W = NQ // 512    # query windows of 512
MT = N // 128    # key m-tiles of 128
NGROUPS = 8
EPS = 1e-5
GELEM = (C // NGROUPS) * N   # elements per norm group
NGRP = 11                    # m-groups per head stream: [3]*10 + [2]

_COMPILED = None


def _emit(tc, io):
    import concourse.bass as bass
    from concourse import mybir
    from contextlib import ExitStack

    nc = tc.nc
    f32 = mybir.dt.float32
    bf16 = mybir.dt.bfloat16
    Alu = mybir.AluOpType
    Act = mybir.ActivationFunctionType

    xb, qkvw, qkvb, projw, projb, nw, nb, y = (
        io["xb"], io["qkvw"], io["qkvb"], io["projw"], io["projb"],
        io["nw"], io["nb"], io["y"])

    ctx = ExitStack()
    with ctx:
        # ---------------- pools ----------------
        # PSUM: pool A (3 banks) = even-head S stream, pool B (3 banks) =
        # odd-head S stream, pv pool 2x1 bank. 3+3+2 = 8 banks. Phase 1/3/5
        # transposes/projection chains borrow A/B between attention uses.
        left = ctx.enter_context(tc.tile_pool(name="left", bufs=1))
        psum_a = ctx.enter_context(tc.tile_pool(name="psum_a", bufs=1, space="PSUM"))
        psum_b = ctx.enter_context(tc.tile_pool(name="psum_b", bufs=1, space="PSUM"))
        psum_pv = ctx.enter_context(tc.tile_pool(name="psum_pv", bufs=2, space="PSUM"))
        pool_ab = [psum_a, psum_b]

        right_ctx = ExitStack()
        xf_pool = right_ctx.enter_context(
            tc.tile_pool(name="xf_pool", bufs=1, side="right"))
        wstg_pool = right_ctx.enter_context(
            tc.tile_pool(name="wstg_pool", bufs=4, side="right"))
        scr_pool = right_ctx.enter_context(
            tc.tile_pool(name="scr_pool", bufs=2, side="right"))

        # ---------------- persistent tiles ----------------
        xn = [left.tile([128, N], bf16, name=f"xn{t}", tag=f"xn{t}") for t in range(T)]
        ksb = [left.tile([128, N], bf16, name=f"ksb{t}", tag=f"ksb{t}") for t in range(T)]
        qsb = [left.tile([128, NQ], bf16, name=f"qsb{t}", tag=f"qsb{t}") for t in range(T)]
        wTq = [left.tile([128, 1536], bf16, name=f"wTq{t}", tag=f"wTq{t}") for t in range(T)]
        wTp = [left.tile([128, C], bf16, name=f"wTp{t}", tag=f"wTp{t}") for t in range(T)]
        vb_bc = left.tile([128, C], f32, name="vb_bc", tag="vb_bc")
        ones_row = left.tile([1, D], f32, name="ones_row", tag="ones_row")
        qb = [left.tile([128, 1], f32, name=f"qb{i}", tag=f"qb{i}") for i in range(8)]
        pb = [left.tile([128, 1], f32, name=f"pb{i}", tag=f"pb{i}") for i in range(T)]
        nwt = [left.tile([128, 1], f32, name=f"nwt{t}", tag=f"nwt{t}") for t in range(T)]
        nbt = [left.tile([128, 1], f32, name=f"nbt{t}", tag=f"nbt{t}") for t in range(T)]
        stat = [left.tile([128, 2], f32, name=f"stat{t}", tag=f"stat{t}") for t in range(T)]
        gstat = [left.tile([128, 2], f32, name=f"gstat{t}", tag=f"gstat{t}") for t in range(T)]

        # ---------------- input DMAs ----------------
        xf = [xf_pool.tile([128, N], f32, name=f"xf{t}", tag=f"xf{t}") for t in range(T)]
        for t in range(T):
            for c4 in range(4):   # split across DMA queues
                nc.sync.dma_start(
                    xf[t][:, 1024 * c4:1024 * (c4 + 1)],
                    xb[128 * t:128 * (t + 1), 1024 * c4:1024 * (c4 + 1)])
            nc.sync.dma_start(nwt[t][:, 0:1], nw[128 * t:128 * (t + 1)])
            nc.sync.dma_start(nbt[t][:, 0:1], nb[128 * t:128 * (t + 1)])
            nc.sync.dma_start(pb[t][:, 0:1], projb[128 * t:128 * (t + 1)])
        for i in range(8):
            nc.sync.dma_start(qb[i][:, 0:1], qkvb[128 * i:128 * (i + 1)])
        # v bias broadcast to 128 partitions (stride-0 partition read)
        nc.gpsimd.dma_start(
            out=vb_bc[:],
            in_=bass.AP(tensor=qkvb.tensor, offset=1024, ap=[[0, 128], [1, C]]))
        nc.vector.memset(ones_row[0:1, :], 1.0)

        # weights: natural-layout contiguous DMA, cast to bf16, transpose
        # 128x128 blocks on the PE (identity trick) into wTq/wTp.
        ident = left.tile([128, 128], bf16, name="ident", tag="ident")
        nc.sync.dma_start(ident[:], io["cid"][:, :])
        ind = left.tile([128, 2], f32, name="ind", tag="ind")
        nc.sync.dma_start(ind[:], io["cind"][:, :])
        indT = left.tile([2, 128], f32, name="indT", tag="indT")
        nc.sync.dma_start(indT[0:2, :], io["cindT"][:, :])
        for i in range(12):   # qkv_w row-tiles
            wstg = wstg_pool.tile([128, C], f32, name="wstg", tag="wstg")
            nc.sync.dma_start(wstg[:], qkvw[128 * i:128 * (i + 1), :])
            wbf = wstg_pool.tile([128, C], bf16, name="wbf", tag="wbf")
            nc.vector.tensor_copy(wbf[:], wstg[:])
            for j in range(T):
                tp = pool_ab[i % 2].tile([128, 128], bf16, name="tp", tag="sA" if i % 2 == 0 else "sB")
                nc.tensor.transpose(tp[:], wbf[:, 128 * j:128 * (j + 1)], ident[:])
                nc.vector.tensor_copy(wTq[j][:, 128 * i:128 * (i + 1)], tp[:])
        for i in range(4):    # proj_w row-tiles
            wstg = wstg_pool.tile([128, C], f32, name="wstg", tag="wstg")
            nc.sync.dma_start(wstg[:], projw[128 * i:128 * (i + 1), :])
            wbf = wstg_pool.tile([128, C], bf16, name="wbf", tag="wbf")
            nc.vector.tensor_copy(wbf[:], wstg[:])
            for j in range(T):
                tp = pool_ab[i % 2].tile([128, 128], bf16, name="tp", tag="sA" if i % 2 == 0 else "sB")
                nc.tensor.transpose(tp[:], wbf[:, 128 * j:128 * (j + 1)], ident[:])
                nc.vector.tensor_copy(wTp[j][:, 128 * i:128 * (i + 1)], tp[:])

        # ---------------- phase 1: group stats ----------------
        for t in range(T):
            nc.vector.tensor_reduce(
                out=stat[t][:, 0:1], in_=xf[t][:], axis=mybir.AxisListType.X, op=Alu.add)
            sq_scr = scr_pool.tile([128, N], bf16, name="sq_scr", tag="sq_scr")
            nc.scalar.activation(
                sq_scr[:], xf[t][:], Act.Square, accum_out=stat[t][:, 1:2])
            # group-reduce over partitions via indicator matmuls:
            # gg[g,s] = sum_ch ind[ch,g]*stat[ch,s]; then broadcast back
            gg_ps = psum_a.tile([2, 2], f32, name="gg_ps", tag="sA")
            nc.tensor.matmul(gg_ps[0:2, :], ind[:, 0:2], stat[t][:, 0:2],
                             start=True, stop=True)
            gg_sb = left.tile([2, 2], f32, name=f"gg_sb{t}", tag=f"gg_sb{t}")
            nc.vector.tensor_copy(gg_sb[0:2, :], gg_ps[0:2, :])
            gb_ps = psum_b.tile([128, 2], f32, name="gb_ps", tag="sB")
            nc.tensor.matmul(gb_ps[:, 0:2], indT[0:2, :], gg_sb[0:2, :],
                             start=True, stop=True)
            nc.vector.tensor_copy(gstat[t][:, 0:2], gb_ps[:, 0:2])
            # mean/var/rstd -> per-channel affine a,b
            mean_t = left.tile([128, 1], f32, name=f"mean{t}", tag=f"mean{t}")
            e2_t = left.tile([128, 1], f32, name=f"e2{t}", tag=f"e2{t}")
            var_t = left.tile([128, 1], f32, name=f"var{t}", tag=f"var{t}")
            std_t = left.tile([128, 1], f32, name=f"std{t}", tag=f"std{t}")
            a_t = left.tile([128, 1], f32, name=f"a{t}", tag=f"a{t}")
            b_t = left.tile([128, 1], f32, name=f"b{t}", tag=f"b{t}")
            inv = 1.0 / GELEM
            nc.vector.tensor_scalar(mean_t[:], gstat[t][:, 0:1], inv, None, Alu.mult)
            nc.vector.tensor_scalar(e2_t[:], gstat[t][:, 1:2], inv, None, Alu.mult)
            nc.vector.scalar_tensor_tensor(
                var_t[:], mean_t[:], -1.0, mean_t[:], Alu.mult, Alu.mult)
            nc.vector.scalar_tensor_tensor(
                var_t[:], e2_t[:], EPS, var_t[:], Alu.add, Alu.add)
            nc.scalar.activation(std_t[:], var_t[:], Act.Sqrt)
            nc.vector.reciprocal(a_t[:], std_t[:])
            nc.vector.tensor_tensor(a_t[:], a_t[:], nwt[t][:], Alu.mult)
            nc.vector.tensor_tensor(b_t[:], mean_t[:], a_t[:], Alu.mult)
            nc.vector.tensor_tensor(b_t[:], nbt[t][:], b_t[:], Alu.subtract)
            # phase 2: normalize + cast
            nc.vector.tensor_scalar(
                xn[t][:], xf[t][:], a_t[:, 0:1], b_t[:, 0:1], Alu.mult, Alu.add)

        right_ctx.close()

        # ---------------- mid pools (reuse xf space) ----------------
        mid = ctx.enter_context(tc.tile_pool(name="mid", bufs=1))
        psb_pool = ctx.enter_context(tc.tile_pool(name="psb_pool", bufs=4))
        rec_pool = ctx.enter_context(tc.tile_pool(name="rec_pool", bufs=2))
        yo_pool = ctx.enter_context(tc.tile_pool(name="yo_pool", bufs=2))

        vT = mid.tile([128, MT * 520], bf16, name="vT", tag="vT")
        attn = [mid.tile([128, NQ], bf16, name=f"attn{t}", tag=f"attn{t}") for t in range(T)]
        xres = [mid.tile([128, NQ], f32, name=f"xres{t}", tag=f"xres{t}") for t in range(T)]
        for t in range(T):
            nc.sync.dma_start(xres[t][:], xb[128 * t:128 * (t + 1), 0:NQ])

        # ones columns of the augmented v^T (denominator trick)
        ones_view = vT[:].rearrange("p (m h x) -> p m h x", m=MT, x=65)[:, :, :, 64:65]
        nc.vector.memset(ones_view, 1.0)

        # ---------------- phase 3: projections ----------------
        # q: out rows 0..511 of qkv, only window-0 columns here; window-1 q
        # and k tiles 1..3 are emitted later as attention-period fillers.
        def q_chain(i, w, par):
            qp = pool_ab[par].tile([128, 512], f32, name="qp",
                                   tag="sA" if par == 0 else "sB")
            for k in range(T):
                nc.tensor.matmul(
                    qp[:], wTq[k][:, 128 * i:128 * i + 128],
                    xn[k][:, 512 * w:512 * w + 512],
                    start=(k == 0), stop=(k == T - 1))
            nc.vector.tensor_scalar(
                qsb[i][:, 512 * w:512 * w + 512], qp[:], qb[i][:, 0:1], None, Alu.add)

        def k_chain(i, w, par):
            kp = pool_ab[par].tile([128, 512], f32, name="kp",
                                   tag="sA" if par == 0 else "sB")
            for k in range(T):
                nc.tensor.matmul(
                    kp[:], wTq[k][:, 512 + 128 * i:512 + 128 * i + 128],
                    xn[k][:, 512 * w:512 * w + 512],
                    start=(k == 0), stop=(k == T - 1))
            nc.vector.tensor_scalar(
                ksb[i][:, 512 * w:512 * w + 512], kp[:], qb[4 + i][:, 0:1], None, Alu.add)

        def v_chain(mt, par):
            vp = pool_ab[par].tile([128, 512], f32, name="vp",
                                   tag="sA" if par == 0 else "sB")
            for k in range(T):
                nc.tensor.matmul(
                    vp[:], xn[k][:, 128 * mt:128 * mt + 128],
                    wTq[k][:, 1024:1536],
                    start=(k == 0), stop=(k == T - 1))
            dst = vT[:, 520 * mt:520 * mt + 520].rearrange(
                "p (h x) -> p h x", x=65)[:, :, 0:64]
            srcv = vp[:].rearrange("p (h x) -> p h x", x=64)
            vbv = vb_bc[:].rearrange("p (h x) -> p h x", x=64)
            nc.vector.tensor_tensor(dst, srcv, vbv, Alu.add)

        def proj_chain(i, w, par):
            py = pool_ab[par].tile([128, 512], f32, name="py",
                                   tag="sA" if par == 0 else "sB")
            for k in range(T):
                nc.tensor.matmul(
                    py[:], wTp[k][:, 128 * i:128 * i + 128],
                    attn[k][:, 512 * w:512 * w + 512],
                    start=(k == 0), stop=(k == T - 1))
            yo = yo_pool.tile([128, 512], f32, name="yo", tag="yo")
            nc.vector.scalar_tensor_tensor(
                yo[:], py[:], pb[i][:, 0:1], xres[i][:, 512 * w:512 * w + 512],
                Alu.add, Alu.add)
            nc.sync.dma_start(y[128 * i:128 * i + 128, 512 * w:512 * w + 512], yo[:])

        # prefix: k tile 0, window-0 q, all of vT (PV consumes vT from the
        # first attention period on)
        for w8 in range(8):
            k_chain(0, w8, w8 % 2)
        for i in range(T):
            q_chain(i, 0, i % 2)
        for mt in range(MT):
            v_chain(mt, mt % 2)

        # ---------------- phase 4: attention (flat pipelined stream) ------
        # Global stream of periods over (window, pair, group). PV runs one
        # period behind S/exp; pair normalize is deferred into the next
        # pair's first period; filler chains (k tiles 1-3, window-1 q,
        # window-0 proj) are emitted on alternate periods.
        def gsize(r):
            return 3 if r < NGRP - 1 else MT - 3 * (NGRP - 1)

        periods = [(w, p, r) for w in range(W) for p in range(4)
                   for r in range(NGRP)]
        pair_state = {}

        def emit_pv(w, p, r):
            pvs, ps_t = pair_state[(w, p)]
            if pvs[0] is None:
                for hh in range(2):
                    pvs[hh] = psum_pv.tile([128, 512], f32, name=f"pv{hh}", tag="pv")
            gs = gsize(r)
            for hh in range(2):
                h = 2 * p + hh
                pst = ps_t[hh][r]
                for j in range(gs):
                    m = 3 * r + j
                    nc.tensor.matmul(
                        pvs[hh][0:65, :],
                        vT[:, 520 * m + 65 * h:520 * m + 65 * h + 65],
                        pst[:, 512 * j:512 * j + 512],
                        start=(m == 0), stop=(m == MT - 1))

        def emit_normalize(w, p):
            pvs, _ = pair_state[(w, p)]
            for hh in range(2):
                h = 2 * p + hh
                kt, prr = h // 2, 64 * (h % 2)
                dnm = rec_pool.tile([1, 512], f32, name="dnm", tag="dnm")
                nc.vector.tensor_copy(dnm[0:1, :], pvs[hh][64:65, :])
                rec = rec_pool.tile([1, 512], f32, name="rec", tag="rec")
                rscr = rec_pool.tile([1, 512], f32, name="rscr", tag="rscr")
                nc.vector.reciprocal_approx_accurate(
                    rec[0:1, :], dnm[0:1, :], rscr[0:1, :])
                bc = pool_ab[hh].tile([64, 512], f32, name="bc",
                                     tag="sA" if hh == 0 else "sB")
                nc.tensor.matmul(
                    bc[0:64, :], ones_row[0:1, 0:D],
                    rec[0:1, :], start=True, stop=True)
                bcs = rec_pool.tile([64, 512], f32, name="bcs", tag="bcs")
                nc.vector.tensor_copy(bcs[0:64, :], bc[0:64, :])
                nc.vector.tensor_tensor(
                    attn[kt][prr:prr + 64, 512 * w:512 * w + 512],
                    pvs[hh][0:64, :], bcs[0:64, :], Alu.mult)

        # filler schedule: (earliest_period, closure); one pop per period.
        # Thresholds guarantee each chain is EMITTED before any consumer:
        # ksb[i] needed from period 11*i, window-1 q from period 44,
        # window-0 proj readable after normalize(w0,p3) lands at period 44.
        fillers = []
        for i in range(1, T):                       # ksb[1..3]: pops 8i-7..8i
            for w8 in range(8):
                fillers.append((8 * (i - 1) + 1 + w8, lambda i=i, w8=w8, par=w8 % 2:
                                k_chain(i, w8, par)))
        for i in range(T):                          # window-1 q: pops 25..28
            fillers.append((25 + i, lambda i=i, par=i % 2: q_chain(i, 1, par)))
        for i in range(T):                          # window-0 proj: pops 46..49
            fillers.append((46 + i, lambda i=i, par=i % 2: proj_chain(i, 0, par)))
        fillers.reverse()   # pop from the end

        for g, (w, p, r) in enumerate(periods):
            gs = gsize(r)
            if r == 0:
                pair_state[(w, p)] = (
                    [None, None], [[None] * NGRP, [None] * NGRP])
            pvs, ps_t = pair_state[(w, p)]
            for hh in range(2):
                pr = 64 * hh
                sp = pool_ab[hh].tile([128, 512 * gs], f32, name=f"sp{hh}",
                                      tag="sA" if hh == 0 else "sB")
                for j in range(gs):
                    m = 3 * r + j
                    nc.tensor.matmul(
                        sp[:, 512 * j:512 * j + 512],
                        ksb[p][pr:pr + 64, 128 * m:128 * m + 128],
                        qsb[p][pr:pr + 64, 512 * w:512 * w + 512],
                        start=True, stop=True)
                pst = psb_pool.tile([128, 1536], bf16, name="ps", tag="ps")
                nc.scalar.activation(
                    pst[:, 0:512 * gs], sp[:, 0:512 * gs], Act.Exp, scale=0.125)
                ps_t[hh][r] = pst
                # PV of the previous period goes between the two S blocks
                # (h0 part) and after them (h1 part)
                if hh == 0:
                    if g > 0:
                        pw, pp, prr_ = periods[g - 1]
                        emit_pv(pw, pp, prr_)
            if g > 0 and periods[g - 1][2] == NGRP - 1:
                emit_normalize(*periods[g - 1][:2])
            if fillers and fillers[-1][0] <= g:
                fillers.pop()[1]()

        # tail: last period's PV, last normalize, window-1 proj
        emit_pv(*periods[-1])
        emit_normalize(W - 1, 3)
        while fillers:
            fillers.pop()[1]()
        for i in range(T):
            proj_chain(i, 1, i % 2)


def _build():
    import concourse.tile as tile
    from concourse import bacc, mybir

    nc = bacc.Bacc("TRN2", target_bir_lowering=False, debug=False)
    f32 = mybir.dt.float32
    io = {
        "xb": nc.dram_tensor("xb", [C, N], f32, kind="ExternalInput").ap(),
        "qkvw": nc.dram_tensor("qkvw", [3 * C, C], f32, kind="ExternalInput").ap(),
        "qkvb": nc.dram_tensor("qkvb", [3 * C], f32, kind="ExternalInput").ap(),
        "projw": nc.dram_tensor("projw", [C, C], f32, kind="ExternalInput").ap(),
        "projb": nc.dram_tensor("projb", [C], f32, kind="ExternalInput").ap(),
        "nw": nc.dram_tensor("nw", [C], f32, kind="ExternalInput").ap(),
        "nb": nc.dram_tensor("nb", [C], f32, kind="ExternalInput").ap(),
        "cid": nc.dram_tensor("cid", [128, 128], mybir.dt.bfloat16,
                              kind="ExternalInput").ap(),
        "cind": nc.dram_tensor("cind", [128, 2], f32, kind="ExternalInput").ap(),
        "cindT": nc.dram_tensor("cindT", [2, 128], f32, kind="ExternalInput").ap(),
        "y": nc.dram_tensor("y", [C, NQ], f32, kind="ExternalOutput").ap(),
    }
    with tile.TileContext(nc) as tc:
        _emit(tc, io)
    nc.compile()
    return nc


def get_compiled():
    global _COMPILED
    if _COMPILED is None:
        _COMPILED = _build()
    return _COMPILED


def make_in_maps(x, norm_w, norm_b, qkv_w, qkv_b, proj_w, proj_b):
    import ml_dtypes

    xf = np.ascontiguousarray(np.asarray(x, np.float32)).reshape(2, C, N)
    ind = np.zeros((128, 2), np.float32)
    ind[0:64, 0] = 1.0
    ind[64:128, 1] = 1.0
    shared = {
        "cid": np.eye(128, dtype=ml_dtypes.bfloat16),
        "cind": ind,
        "cindT": np.ascontiguousarray(ind.T),
        "qkvw": np.ascontiguousarray(np.asarray(qkv_w, np.float32)),
        "qkvb": np.ascontiguousarray(np.asarray(qkv_b, np.float32)),
        "projw": np.ascontiguousarray(np.asarray(proj_w, np.float32)),
        "projb": np.ascontiguousarray(np.asarray(proj_b, np.float32)),
        "nw": np.ascontiguousarray(np.asarray(norm_w, np.float32)),
        "nb": np.ascontiguousarray(np.asarray(norm_b, np.float32)),
    }
    in_maps = []
    for core in range(8):
        bi, qs = core // 4, core % 4
        # rotate so this core's queries are always columns [0:NQ)
        xroll = np.concatenate(
            [xf[bi][:, qs * NQ:], xf[bi][:, :qs * NQ]], axis=1)
        m = dict(shared)
        m["xb"] = np.ascontiguousarray(xroll)
        in_maps.append(m)
    return in_maps


def assemble(results, x):
    y = np.zeros((2, C, N), np.float32)
    for core in range(8):
        bi, qs = core // 4, core % 4
        y[bi][:, qs * NQ:(qs + 1) * NQ] = results[core]["y"]
    return y.reshape(x.shape)


def kernel(x, norm_w, norm_b, qkv_w, qkv_b, proj_w, proj_b, **_ignored):
    from concourse import bass_utils

    nc = get_compiled()
    in_maps = make_in_maps(x, norm_w, norm_b, qkv_w, qkv_b, proj_w, proj_b)
    res = bass_utils.run_bass_kernel_spmd(nc, in_maps, core_ids=list(range(8)))
    return assemble(res.results, np.asarray(x))


# revision 7
# speedup vs baseline: 1.5297x; 1.0151x over previous
"""Trainium2 Bass kernel for nn_AttentionBlock (GroupNorm + MHA + proj + residual).

Full inputs in, full output out. Sharding: 8 cores = 2 batches x 4 query-slices.
Each core: GroupNorm over its batch image (replicated within the batch group),
q projection for its 1024 queries, k/v projections over all 4096 keys,
per-head attention (S^T = k^T q formulation, softmax along the PSUM partition
axis via an appended ones-column in the PV matmul), output projection and
residual for its query slice. Host side only slices/rotates/concatenates.

v2: phase 4 is software-pipelined per head-PAIR: the even head's S groups live
in a 3-bank PSUM pool A, the odd head's in pool B (plus 2 PV banks = 8).
Softmax exp runs as one N=1536 activation per group so ScalarE (the kernel's
throughput floor: ~2 exps of 16K elems per query-window per head) streams with
no gaps; PV matmuls are emitted one period behind S so the tensor engine FIFO
never stalls behind an exp dependency.

All matmuls run in bf16 with fp32 PSUM accumulation; softmax logits stay fp32.
"""
import numpy as np

C = 512          # channels
N = 4096         # pixels (64*64)
NQ = 1024        # queries per core
H = 8            # heads
D = 64           # head dim
T = 4            # 128-channel chunks
W = NQ // 512    # query windows of 512
MT = N // 128    # key m-tiles of 128
NGROUPS = 8
EPS = 1e-5
GELEM = (C // NGROUPS) * N   # elements per norm group
NGRP = 11                    # m-groups per head stream: [3]*10 + [2]

_COMPILED = None


def _emit(tc, io):
    import concourse.bass as bass
    from concourse import mybir
    from contextlib import ExitStack

    nc = tc.nc
    f32 = mybir.dt.float32
    bf16 = mybir.dt.bfloat16
    Alu = mybir.AluOpType
    Act = mybir.ActivationFunctionType

    xb, qkvw, qkvb, projw, projb, nw, nb, y = (
        io["xb"], io["qkvw"], io["qkvb"], io["projw"], io["projb"],
        io["nw"], io["nb"], io["y"])

    ctx = ExitStack()
    with ctx:
        # ---------------- pools ----------------
        # PSUM: pool A (3 banks) = even-head S stream, pool B (3 banks) =
        # odd-head S stream, pv pool 2x1 bank. 3+3+2 = 8 banks. Phase 1/3/5
        # transposes/projection chains borrow A/B between attention uses.
        left = ctx.enter_context(tc.tile_pool(name="left", bufs=1))
        psum_a = ctx.enter_context(tc.tile_pool(name="psum_a", bufs=1, space="PSUM"))
        psum_b = ctx.enter_context(tc.tile_pool(name="psum_b", bufs=1, space="PSUM"))
        psum_pv = ctx.enter_context(tc.tile_pool(name="psum_pv", bufs=2, space="PSUM"))
        pool_ab = [psum_a, psum_b]

        right_ctx = ExitStack()
        xf_pool = right_ctx.enter_context(
            tc.tile_pool(name="xf_pool", bufs=1, side="right"))
        wstg_pool = right_ctx.enter_context(
            tc.tile_pool(name="wstg_pool", bufs=4, side="right"))
        scr_pool = right_ctx.enter_context(
            tc.tile_pool(name="scr_pool", bufs=2, side="right"))

        # ---------------- persistent tiles ----------------
        xn = [left.tile([128, N], bf16, name=f"xn{t}", tag=f"xn{t}") for t in range(T)]
        ksb = [left.tile([128, N], bf16, name=f"ksb{t}", tag=f"ksb{t}") for t in range(T)]
        qsb = [left.tile([128, NQ], bf16, name=f"qsb{t}", tag=f"qsb{t}") for t in range(T)]
        wTq = [left.tile([128, 1536], bf16, name=f"wTq{t}", tag=f"wTq{t}") for t in range(T)]
        wTp = [left.tile([128, C], bf16, name=f"wTp{t}", tag=f"wTp{t}") for t in range(T)]
        vb_bc = left.tile([128, C], f32, name="vb_bc", tag="vb_bc")
        ones_row = left.tile([1, D], f32, name="ones_row", tag="ones_row")
        qb = [left.tile([128, 1], f32, name=f"qb{i}", tag=f"qb{i}") for i in range(8)]
        pb = [left.tile([128, 1], f32, name=f"pb{i}", tag=f"pb{i}") for i in range(T)]
        nwt = [left.tile([128, 1], f32, name=f"nwt{t}", tag=f"nwt{t}") for t in range(T)]
        nbt = [left.tile([128, 1], f32, name=f"nbt{t}", tag=f"nbt{t}") for t in range(T)]
        stat = [left.tile([128, 2], f32, name=f"stat{t}", tag=f"stat{t}") for t in range(T)]
        gstat = [left.tile([128, 2], f32, name=f"gstat{t}", tag=f"gstat{t}") for t in range(T)]

        # weights: natural-layout contiguous DMA, cast to bf16, transpose
        # 128x128 blocks on the PE (identity trick) into wTq/wTp.
        ident = left.tile([128, 128], bf16, name="ident", tag="ident")
        nc.sync.dma_start(ident[:], io["cid"][:, :])
        ind = left.tile([128, 2], f32, name="ind", tag="ind")
        nc.sync.dma_start(ind[:], io["cind"][:, :])
        indT = left.tile([2, 128], f32, name="indT", tag="indT")
        nc.sync.dma_start(indT[0:2, :], io["cindT"][:, :])
        for i in range(12):   # qkv_w row-tiles
            wstg = wstg_pool.tile([128, C], f32, name="wstg", tag="wstg")
            nc.sync.dma_start(wstg[:], qkvw[128 * i:128 * (i + 1), :])
            wbf = wstg_pool.tile([128, C], bf16, name="wbf", tag="wbf")
            nc.vector.tensor_copy(wbf[:], wstg[:])
            for j in range(T):
                tp = pool_ab[i % 2].tile([128, 128], bf16, name="tp", tag="sA" if i % 2 == 0 else "sB")
                nc.tensor.transpose(tp[:], wbf[:, 128 * j:128 * (j + 1)], ident[:])
                nc.vector.tensor_copy(wTq[j][:, 128 * i:128 * (i + 1)], tp[:])
        for i in range(4):    # proj_w row-tiles
            wstg = wstg_pool.tile([128, C], f32, name="wstg", tag="wstg")
            nc.sync.dma_start(wstg[:], projw[128 * i:128 * (i + 1), :])
            wbf = wstg_pool.tile([128, C], bf16, name="wbf", tag="wbf")
            nc.vector.tensor_copy(wbf[:], wstg[:])
            for j in range(T):
                tp = pool_ab[i % 2].tile([128, 128], bf16, name="tp", tag="sA" if i % 2 == 0 else "sB")
                nc.tensor.transpose(tp[:], wbf[:, 128 * j:128 * (j + 1)], ident[:])
                nc.vector.tensor_copy(wTp[j][:, 128 * i:128 * (i + 1)], tp[:])

        # ---------------- input DMAs ----------------
        xf = [xf_pool.tile([128, N], f32, name=f"xf{t}", tag=f"xf{t}") for t in range(T)]
        for t in range(T):
            for c4 in range(4):   # split across DMA queues
                eng = nc.sync if c4 % 2 == 0 else nc.scalar
                eng.dma_start(
                    xf[t][:, 1024 * c4:1024 * (c4 + 1)],
                    xb[128 * t:128 * (t + 1), 1024 * c4:1024 * (c4 + 1)])
            nc.sync.dma_start(nwt[t][:, 0:1], nw[128 * t:128 * (t + 1)])
            nc.sync.dma_start(nbt[t][:, 0:1], nb[128 * t:128 * (t + 1)])
            nc.sync.dma_start(pb[t][:, 0:1], projb[128 * t:128 * (t + 1)])
        for i in range(8):
            nc.sync.dma_start(qb[i][:, 0:1], qkvb[128 * i:128 * (i + 1)])
        # v bias broadcast to 128 partitions (stride-0 partition read)
        nc.gpsimd.dma_start(
            out=vb_bc[:],
            in_=bass.AP(tensor=qkvb.tensor, offset=1024, ap=[[0, 128], [1, C]]))
        nc.vector.memset(ones_row[0:1, :], 1.0)

        # ---------------- phase 1: group stats ----------------
        for t in range(T):
            nc.vector.tensor_reduce(
                out=stat[t][:, 0:1], in_=xf[t][:], axis=mybir.AxisListType.X, op=Alu.add)
            sq_scr = scr_pool.tile([128, N], bf16, name="sq_scr", tag="sq_scr")
            nc.scalar.activation(
                sq_scr[:], xf[t][:], Act.Square, accum_out=stat[t][:, 1:2])
            # group-reduce over partitions via indicator matmuls:
            # gg[g,s] = sum_ch ind[ch,g]*stat[ch,s]; then broadcast back
            gg_ps = psum_a.tile([2, 2], f32, name="gg_ps", tag="sA")
            nc.tensor.matmul(gg_ps[0:2, :], ind[:, 0:2], stat[t][:, 0:2],
                             start=True, stop=True)
            gg_sb = left.tile([2, 2], f32, name=f"gg_sb{t}", tag=f"gg_sb{t}")
            nc.vector.tensor_copy(gg_sb[0:2, :], gg_ps[0:2, :])
            gb_ps = psum_b.tile([128, 2], f32, name="gb_ps", tag="sB")
            nc.tensor.matmul(gb_ps[:, 0:2], indT[0:2, :], gg_sb[0:2, :],
                             start=True, stop=True)
            nc.vector.tensor_copy(gstat[t][:, 0:2], gb_ps[:, 0:2])
            # mean/var/rstd -> per-channel affine a,b
            mean_t = left.tile([128, 1], f32, name=f"mean{t}", tag=f"mean{t}")
            e2_t = left.tile([128, 1], f32, name=f"e2{t}", tag=f"e2{t}")
            var_t = left.tile([128, 1], f32, name=f"var{t}", tag=f"var{t}")
            std_t = left.tile([128, 1], f32, name=f"std{t}", tag=f"std{t}")
            a_t = left.tile([128, 1], f32, name=f"a{t}", tag=f"a{t}")
            b_t = left.tile([128, 1], f32, name=f"b{t}", tag=f"b{t}")
            inv = 1.0 / GELEM
            nc.vector.tensor_scalar(mean_t[:], gstat[t][:, 0:1], inv, None, Alu.mult)
            nc.vector.tensor_scalar(e2_t[:], gstat[t][:, 1:2], inv, None, Alu.mult)
            nc.vector.scalar_tensor_tensor(
                var_t[:], mean_t[:], -1.0, mean_t[:], Alu.mult, Alu.mult)
            nc.vector.scalar_tensor_tensor(
                var_t[:], e2_t[:], EPS, var_t[:], Alu.add, Alu.add)
            nc.scalar.activation(std_t[:], var_t[:], Act.Sqrt)
            nc.vector.reciprocal(a_t[:], std_t[:])
            nc.vector.tensor_tensor(a_t[:], a_t[:], nwt[t][:], Alu.mult)
            nc.vector.tensor_tensor(b_t[:], mean_t[:], a_t[:], Alu.mult)
            nc.vector.tensor_tensor(b_t[:], nbt[t][:], b_t[:], Alu.subtract)
            # phase 2: normalize + cast
            nc.vector.tensor_scalar(
                xn[t][:], xf[t][:], a_t[:, 0:1], b_t[:, 0:1], Alu.mult, Alu.add)

        right_ctx.close()

        # ---------------- mid pools (reuse xf space) ----------------
        mid = ctx.enter_context(tc.tile_pool(name="mid", bufs=1))
        psb_pool = ctx.enter_context(tc.tile_pool(name="psb_pool", bufs=4))
        rec_pool = ctx.enter_context(tc.tile_pool(name="rec_pool", bufs=2))
        yo_pool = ctx.enter_context(tc.tile_pool(name="yo_pool", bufs=2))

        vT = mid.tile([128, MT * 520], bf16, name="vT", tag="vT")
        attn = [mid.tile([128, NQ], bf16, name=f"attn{t}", tag=f"attn{t}") for t in range(T)]
        xres = [mid.tile([128, NQ], f32, name=f"xres{t}", tag=f"xres{t}") for t in range(T)]
        for t in range(T):
            nc.sync.dma_start(xres[t][:], xb[128 * t:128 * (t + 1), 0:NQ])

        # ones columns of the augmented v^T (denominator trick)
        ones_view = vT[:].rearrange("p (m h x) -> p m h x", m=MT, x=65)[:, :, :, 64:65]
        nc.vector.memset(ones_view, 1.0)

        # ---------------- phase 3: projections ----------------
        # q: out rows 0..511 of qkv, only window-0 columns here; window-1 q
        # and k tiles 1..3 are emitted later as attention-period fillers.
        def q_chain(i, w, par):
            qp = pool_ab[par].tile([128, 512], f32, name="qp",
                                   tag="sA" if par == 0 else "sB")
            for k in range(T):
                nc.tensor.matmul(
                    qp[:], wTq[k][:, 128 * i:128 * i + 128],
                    xn[k][:, 512 * w:512 * w + 512],
                    start=(k == 0), stop=(k == T - 1))
            nc.vector.tensor_scalar(
                qsb[i][:, 512 * w:512 * w + 512], qp[:], qb[i][:, 0:1], None, Alu.add)

        def k_chain(i, w, par):
            kp = pool_ab[par].tile([128, 512], f32, name="kp",
                                   tag="sA" if par == 0 else "sB")
            for k in range(T):
                nc.tensor.matmul(
                    kp[:], wTq[k][:, 512 + 128 * i:512 + 128 * i + 128],
                    xn[k][:, 512 * w:512 * w + 512],
                    start=(k == 0), stop=(k == T - 1))
            nc.vector.tensor_scalar(
                ksb[i][:, 512 * w:512 * w + 512], kp[:], qb[4 + i][:, 0:1], None, Alu.add)

        def v_chain(mt, par):
            vp = pool_ab[par].tile([128, 512], f32, name="vp",
                                   tag="sA" if par == 0 else "sB")
            for k in range(T):
                nc.tensor.matmul(
                    vp[:], xn[k][:, 128 * mt:128 * mt + 128],
                    wTq[k][:, 1024:1536],
                    start=(k == 0), stop=(k == T - 1))
            dst = vT[:, 520 * mt:520 * mt + 520].rearrange(
                "p (h x) -> p h x", x=65)[:, :, 0:64]
            srcv = vp[:].rearrange("p (h x) -> p h x", x=64)
            vbv = vb_bc[:].rearrange("p (h x) -> p h x", x=64)
            nc.vector.tensor_tensor(dst, srcv, vbv, Alu.add)

        def proj_chain(i, w, par):
            py = pool_ab[par].tile([128, 512], f32, name="py",
                                   tag="sA" if par == 0 else "sB")
            for k in range(T):
                nc.tensor.matmul(
                    py[:], wTp[k][:, 128 * i:128 * i + 128],
                    attn[k][:, 512 * w:512 * w + 512],
                    start=(k == 0), stop=(k == T - 1))
            yo = yo_pool.tile([128, 512], f32, name="yo", tag="yo")
            nc.vector.scalar_tensor_tensor(
                yo[:], py[:], pb[i][:, 0:1], xres[i][:, 512 * w:512 * w + 512],
                Alu.add, Alu.add)
            nc.sync.dma_start(y[128 * i:128 * i + 128, 512 * w:512 * w + 512], yo[:])

        # prefix: k tiles 0-1, window-0 q, all of vT (PV consumes vT from
        # the first attention period on)
        for w8 in range(8):
            k_chain(0, w8, w8 % 2)
            k_chain(1, w8, (w8 + 1) % 2)
        for i in range(T):
            q_chain(i, 0, i % 2)
        for mt in range(MT):
            v_chain(mt, mt % 2)

        # ---------------- phase 4: attention (flat pipelined stream) ------
        # Global stream of periods over (window, pair, group). PV runs one
        # period behind S/exp; pair normalize is deferred into the next
        # pair's first period; filler chains (k tiles 1-3, window-1 q,
        # window-0 proj) are emitted on alternate periods.
        def gsize(r):
            return 3 if r < NGRP - 1 else MT - 3 * (NGRP - 1)

        periods = [(w, p, r) for w in range(W) for p in range(4)
                   for r in range(NGRP)]
        pair_state = {}

        def emit_pv(w, p, r):
            pvs, ps_t = pair_state[(w, p)]
            if pvs[0] is None:
                for hh in range(2):
                    pvs[hh] = psum_pv.tile([128, 512], f32, name=f"pv{hh}", tag="pv")
            gs = gsize(r)
            for hh in range(2):
                h = 2 * p + hh
                pst = ps_t[hh][r]
                for j in range(gs):
                    m = 3 * r + j
                    nc.tensor.matmul(
                        pvs[hh][0:65, :],
                        vT[:, 520 * m + 65 * h:520 * m + 65 * h + 65],
                        pst[:, 512 * j:512 * j + 512],
                        start=(m == 0), stop=(m == MT - 1))

        def emit_normalize(w, p):
            pvs, _ = pair_state[(w, p)]
            for hh in range(2):
                h = 2 * p + hh
                kt, prr = h // 2, 64 * (h % 2)
                dnm = rec_pool.tile([1, 512], f32, name="dnm", tag="dnm")
                nc.vector.tensor_copy(dnm[0:1, :], pvs[hh][64:65, :])
                rec = rec_pool.tile([1, 512], f32, name="rec", tag="rec")
                rscr = rec_pool.tile([1, 512], f32, name="rscr", tag="rscr")
                nc.vector.reciprocal_approx_accurate(
                    rec[0:1, :], dnm[0:1, :], rscr[0:1, :])
                bc = pool_ab[hh].tile([64, 512], f32, name="bc",
                                     tag="sA" if hh == 0 else "sB")
                nc.tensor.matmul(
                    bc[0:64, :], ones_row[0:1, 0:D],
                    rec[0:1, :], start=True, stop=True)
                bcs = rec_pool.tile([64, 512], f32, name="bcs", tag="bcs")
                nc.vector.tensor_copy(bcs[0:64, :], bc[0:64, :])
                nc.vector.tensor_tensor(
                    attn[kt][prr:prr + 64, 512 * w:512 * w + 512],
                    pvs[hh][0:64, :], bcs[0:64, :], Alu.mult)

        # filler schedule: (earliest_period, closure); one pop on EVEN
        # periods, always from pool B (its next S alloc has ~2x more slack
        # than pool A's, so the filler's drain never delays the exp stream).
        # Emission deadlines: ksb[2] before period 22, ksb[3] before 33,
        # window-1 q before 44, window-0 proj after normalize(w0,p3) at 44.
        fillers = []
        for i in range(2, T):                       # ksb[2..3]: pops 2..16, 18..32
            for w8 in range(8):
                fillers.append((16 * (i - 2) + 2 + 2 * w8,
                                lambda i=i, w8=w8: k_chain(i, w8, 1)))
        for i in range(T):                          # window-1 q: pops 34..40
            fillers.append((34 + 2 * i, lambda i=i: q_chain(i, 1, 1)))
        for i in range(T):                          # window-0 proj: pops 46..52
            fillers.append((46 + 2 * i, lambda i=i: proj_chain(i, 0, 1)))
        fillers.reverse()   # pop from the end

        for g, (w, p, r) in enumerate(periods):
            gs = gsize(r)
            if r == 0:
                pair_state[(w, p)] = (
                    [None, None], [[None] * NGRP, [None] * NGRP])
            pvs, ps_t = pair_state[(w, p)]
            for hh in range(2):
                pr = 64 * hh
                sp = pool_ab[hh].tile([128, 512 * gs], f32, name=f"sp{hh}",
                                      tag="sA" if hh == 0 else "sB")
                for j in range(gs):
                    m = 3 * r + j
                    nc.tensor.matmul(
                        sp[:, 512 * j:512 * j + 512],
                        ksb[p][pr:pr + 64, 128 * m:128 * m + 128],
                        qsb[p][pr:pr + 64, 512 * w:512 * w + 512],
                        start=True, stop=True)
                pst = psb_pool.tile([128, 1536], bf16, name="ps", tag="ps")
                nc.scalar.activation(
                    pst[:, 0:512 * gs], sp[:, 0:512 * gs], Act.Exp, scale=0.125)
                ps_t[hh][r] = pst
                # PV of the previous period goes between the two S blocks
                # (h0 part) and after them (h1 part)
                if hh == 0:
                    if g > 0:
                        pw, pp, prr_ = periods[g - 1]
                        emit_pv(pw, pp, prr_)
            if g > 0 and periods[g - 1][2] == NGRP - 1:
                emit_normalize(*periods[g - 1][:2])
            if g % 2 == 0 and fillers and fillers[-1][0] <= g:
                fillers.pop()[1]()

        # tail: last period's PV, last normalize, window-1 proj
        emit_pv(*periods[-1])
        emit_normalize(W - 1, 3)
        while fillers:
            fillers.pop()[1]()
        for i in range(T):
            proj_chain(i, 1, i % 2)


def _build():
    import concourse.tile as tile
    from concourse import bacc, mybir

    nc = bacc.Bacc("TRN2", target_bir_lowering=False, debug=False)
    f32 = mybir.dt.float32
    io = {
        "xb": nc.dram_tensor("xb", [C, N], f32, kind="ExternalInput").ap(),
        "qkvw": nc.dram_tensor("qkvw", [3 * C, C], f32, kind="ExternalInput").ap(),
        "qkvb": nc.dram_tensor("qkvb", [3 * C], f32, kind="ExternalInput").ap(),
        "projw": nc.dram_tensor("projw", [C, C], f32, kind="ExternalInput").ap(),
        "projb": nc.dram_tensor("projb", [C], f32, kind="ExternalInput").ap(),
        "nw": nc.dram_tensor("nw", [C], f32, kind="ExternalInput").ap(),
        "nb": nc.dram_tensor("nb", [C], f32, kind="ExternalInput").ap(),
        "cid": nc.dram_tensor("cid", [128, 128], mybir.dt.bfloat16,
                              kind="ExternalInput").ap(),
        "cind": nc.dram_tensor("cind", [128, 2], f32, kind="ExternalInput").ap(),
        "cindT": nc.dram_tensor("cindT", [2, 128], f32, kind="ExternalInput").ap(),
        "y": nc.dram_tensor("y", [C, NQ], f32, kind="ExternalOutput").ap(),
    }
    with tile.TileContext(nc) as tc:
        _emit(tc, io)
    nc.compile()
    return nc


def get_compiled():
    global _COMPILED
    if _COMPILED is None:
        _COMPILED = _build()
    return _COMPILED


def make_in_maps(x, norm_w, norm_b, qkv_w, qkv_b, proj_w, proj_b):
    import ml_dtypes

    xf = np.ascontiguousarray(np.asarray(x, np.float32)).reshape(2, C, N)
    ind = np.zeros((128, 2), np.float32)
    ind[0:64, 0] = 1.0
    ind[64:128, 1] = 1.0
    shared = {
        "cid": np.eye(128, dtype=ml_dtypes.bfloat16),
        "cind": ind,
        "cindT": np.ascontiguousarray(ind.T),
        "qkvw": np.ascontiguousarray(np.asarray(qkv_w, np.float32)),
        "qkvb": np.ascontiguousarray(np.asarray(qkv_b, np.float32)),
        "projw": np.ascontiguousarray(np.asarray(proj_w, np.float32)),
        "projb": np.ascontiguousarray(np.asarray(proj_b, np.float32)),
        "nw": np.ascontiguousarray(np.asarray(norm_w, np.float32)),
        "nb": np.ascontiguousarray(np.asarray(norm_b, np.float32)),
    }
    in_maps = []
    for core in range(8):
        bi, qs = core // 4, core % 4
        # rotate so this core's queries are always columns [0:NQ)
        xroll = np.concatenate(
            [xf[bi][:, qs * NQ:], xf[bi][:, :qs * NQ]], axis=1)
        m = dict(shared)
        m["xb"] = np.ascontiguousarray(xroll)
        in_maps.append(m)
    return in_maps


def assemble(results, x):
    y = np.zeros((2, C, N), np.float32)
    for core in range(8):
        bi, qs = core // 4, core % 4
        y[bi][:, qs * NQ:(qs + 1) * NQ] = results[core]["y"]
    return y.reshape(x.shape)


def kernel(x, norm_w, norm_b, qkv_w, qkv_b, proj_w, proj_b, **_ignored):
    from concourse import bass_utils

    nc = get_compiled()
    in_maps = make_in_maps(x, norm_w, norm_b, qkv_w, qkv_b, proj_w, proj_b)
    res = bass_utils.run_bass_kernel_spmd(nc, in_maps, core_ids=list(range(8)))
    return assemble(res.results, np.asarray(x))


# revision 8
# speedup vs baseline: 1.5398x; 1.0066x over previous
"""Trainium2 Bass kernel for nn_AttentionBlock (GroupNorm + MHA + proj + residual).

Full inputs in, full output out. Sharding: 8 cores = 2 batches x 4 query-slices.
Each core: GroupNorm over its batch image (replicated within the batch group),
q projection for its 1024 queries, k/v projections over all 4096 keys,
per-head attention (S^T = k^T q formulation, softmax along the PSUM partition
axis via an appended ones-column in the PV matmul), output projection and
residual for its query slice. Host side only slices/rotates/concatenates.

v2: phase 4 is software-pipelined per head-PAIR: the even head's S groups live
in a 3-bank PSUM pool A, the odd head's in pool B (plus 2 PV banks = 8).
Softmax exp runs as one N=1536 activation per group so ScalarE (the kernel's
throughput floor: ~2 exps of 16K elems per query-window per head) streams with
no gaps; PV matmuls are emitted one period behind S so the tensor engine FIFO
never stalls behind an exp dependency.

All matmuls run in bf16 with fp32 PSUM accumulation; softmax logits stay fp32.
"""
import numpy as np

C = 512          # channels
N = 4096         # pixels (64*64)
NQ = 1024        # queries per core
H = 8            # heads
D = 64           # head dim
T = 4            # 128-channel chunks
W = NQ // 512    # query windows of 512
MT = N // 128    # key m-tiles of 128
NGROUPS = 8
EPS = 1e-5
GELEM = (C // NGROUPS) * N   # elements per norm group
NGRP = 11                    # m-groups per head stream: [3]*10 + [2]

_COMPILED = None


def _emit(tc, io):
    import concourse.bass as bass
    from concourse import mybir
    from contextlib import ExitStack

    nc = tc.nc
    f32 = mybir.dt.float32
    bf16 = mybir.dt.bfloat16
    Alu = mybir.AluOpType
    Act = mybir.ActivationFunctionType

    xb, qkvw, qkvb, projw, projb, nw, nb, y = (
        io["xb"], io["qkvw"], io["qkvb"], io["projw"], io["projb"],
        io["nw"], io["nb"], io["y"])

    ctx = ExitStack()
    with ctx:
        # ---------------- pools ----------------
        # PSUM: pool A (3 banks) = even-head S stream, pool B (3 banks) =
        # odd-head S stream, pv pool 2x1 bank. 3+3+2 = 8 banks. Phase 1/3/5
        # transposes/projection chains borrow A/B between attention uses.
        left = ctx.enter_context(tc.tile_pool(name="left", bufs=1))
        psum_a = ctx.enter_context(tc.tile_pool(name="psum_a", bufs=1, space="PSUM"))
        psum_b = ctx.enter_context(tc.tile_pool(name="psum_b", bufs=1, space="PSUM"))
        psum_pv = ctx.enter_context(tc.tile_pool(name="psum_pv", bufs=2, space="PSUM"))
        pool_ab = [psum_a, psum_b]

        right_ctx = ExitStack()
        xf_pool = right_ctx.enter_context(
            tc.tile_pool(name="xf_pool", bufs=1, side="right"))
        wstg_pool = right_ctx.enter_context(
            tc.tile_pool(name="wstg_pool", bufs=4, side="right"))
        scr_pool = right_ctx.enter_context(
            tc.tile_pool(name="scr_pool", bufs=2, side="right"))

        # ---------------- persistent tiles ----------------
        xn = [left.tile([128, N], bf16, name=f"xn{t}", tag=f"xn{t}") for t in range(T)]
        ksb = [left.tile([128, N], bf16, name=f"ksb{t}", tag=f"ksb{t}") for t in range(T)]
        qsb = [left.tile([128, NQ], bf16, name=f"qsb{t}", tag=f"qsb{t}") for t in range(T)]
        wTq = [left.tile([128, 1536], bf16, name=f"wTq{t}", tag=f"wTq{t}") for t in range(T)]
        wTp = [left.tile([128, C], bf16, name=f"wTp{t}", tag=f"wTp{t}") for t in range(T)]
        vb_bc = left.tile([128, C], f32, name="vb_bc", tag="vb_bc")
        ones_row = left.tile([1, D], f32, name="ones_row", tag="ones_row")
        qb = [left.tile([128, 1], f32, name=f"qb{i}", tag=f"qb{i}") for i in range(8)]
        pb = [left.tile([128, 1], f32, name=f"pb{i}", tag=f"pb{i}") for i in range(T)]
        nwt = [left.tile([128, 1], f32, name=f"nwt{t}", tag=f"nwt{t}") for t in range(T)]
        nbt = [left.tile([128, 1], f32, name=f"nbt{t}", tag=f"nbt{t}") for t in range(T)]
        stat = [left.tile([128, 2], f32, name=f"stat{t}", tag=f"stat{t}") for t in range(T)]
        gstat = [left.tile([128, 2], f32, name=f"gstat{t}", tag=f"gstat{t}") for t in range(T)]

        # ---------------- input DMAs (x tiles 0-1 first) ----------------
        xf = [xf_pool.tile([128, N], f32, name=f"xf{t}", tag=f"xf{t}") for t in range(T)]
        for t in range(2):
            for c4 in range(4):
                nc.sync.dma_start(
                    xf[t][:, 1024 * c4:1024 * (c4 + 1)],
                    xb[128 * t:128 * (t + 1), 1024 * c4:1024 * (c4 + 1)])
        # weights: natural-layout contiguous DMA, cast to bf16, transpose
        # 128x128 blocks on the PE (identity trick) into wTq/wTp.
        ident = left.tile([128, 128], bf16, name="ident", tag="ident")
        nc.sync.dma_start(ident[:], io["cid"][:, :])
        ind = left.tile([128, 2], f32, name="ind", tag="ind")
        nc.sync.dma_start(ind[:], io["cind"][:, :])
        indT = left.tile([2, 128], f32, name="indT", tag="indT")
        nc.sync.dma_start(indT[0:2, :], io["cindT"][:, :])
        for i in range(12):   # qkv_w row-tiles
            wstg = wstg_pool.tile([128, C], f32, name="wstg", tag="wstg")
            nc.sync.dma_start(wstg[:], qkvw[128 * i:128 * (i + 1), :])
            wbf = wstg_pool.tile([128, C], bf16, name="wbf", tag="wbf")
            nc.vector.tensor_copy(wbf[:], wstg[:])
            for j in range(T):
                tp = pool_ab[i % 2].tile([128, 128], bf16, name="tp", tag="sA" if i % 2 == 0 else "sB")
                nc.tensor.transpose(tp[:], wbf[:, 128 * j:128 * (j + 1)], ident[:])
                nc.vector.tensor_copy(wTq[j][:, 128 * i:128 * (i + 1)], tp[:])
        for i in range(4):    # proj_w row-tiles
            wstg = wstg_pool.tile([128, C], f32, name="wstg", tag="wstg")
            nc.sync.dma_start(wstg[:], projw[128 * i:128 * (i + 1), :])
            wbf = wstg_pool.tile([128, C], bf16, name="wbf", tag="wbf")
            nc.vector.tensor_copy(wbf[:], wstg[:])
            for j in range(T):
                tp = pool_ab[i % 2].tile([128, 128], bf16, name="tp", tag="sA" if i % 2 == 0 else "sB")
                nc.tensor.transpose(tp[:], wbf[:, 128 * j:128 * (j + 1)], ident[:])
                nc.vector.tensor_copy(wTp[j][:, 128 * i:128 * (i + 1)], tp[:])

        # ---------------- input DMAs (x tiles 2-3 + consts) ----------------
        for t in range(2, T):
            for c4 in range(4):
                nc.sync.dma_start(
                    xf[t][:, 1024 * c4:1024 * (c4 + 1)],
                    xb[128 * t:128 * (t + 1), 1024 * c4:1024 * (c4 + 1)])
        for t in range(T):
            nc.sync.dma_start(nwt[t][:, 0:1], nw[128 * t:128 * (t + 1)])
            nc.sync.dma_start(nbt[t][:, 0:1], nb[128 * t:128 * (t + 1)])
            nc.sync.dma_start(pb[t][:, 0:1], projb[128 * t:128 * (t + 1)])
        for i in range(8):
            nc.sync.dma_start(qb[i][:, 0:1], qkvb[128 * i:128 * (i + 1)])
        # v bias broadcast to 128 partitions (stride-0 partition read)
        nc.gpsimd.dma_start(
            out=vb_bc[:],
            in_=bass.AP(tensor=qkvb.tensor, offset=1024, ap=[[0, 128], [1, C]]))
        nc.vector.memset(ones_row[0:1, :], 1.0)

        # ---------------- phase 1: group stats ----------------
        for t in range(T):
            nc.vector.tensor_reduce(
                out=stat[t][:, 0:1], in_=xf[t][:], axis=mybir.AxisListType.X, op=Alu.add)
            sq_scr = scr_pool.tile([128, N], bf16, name="sq_scr", tag="sq_scr")
            nc.scalar.activation(
                sq_scr[:], xf[t][:], Act.Square, accum_out=stat[t][:, 1:2])
            # group-reduce over partitions via indicator matmuls:
            # gg[g,s] = sum_ch ind[ch,g]*stat[ch,s]; then broadcast back
            gg_ps = psum_a.tile([2, 2], f32, name="gg_ps", tag="sA")
            nc.tensor.matmul(gg_ps[0:2, :], ind[:, 0:2], stat[t][:, 0:2],
                             start=True, stop=True)
            gg_sb = left.tile([2, 2], f32, name=f"gg_sb{t}", tag=f"gg_sb{t}")
            nc.vector.tensor_copy(gg_sb[0:2, :], gg_ps[0:2, :])
            gb_ps = psum_b.tile([128, 2], f32, name="gb_ps", tag="sB")
            nc.tensor.matmul(gb_ps[:, 0:2], indT[0:2, :], gg_sb[0:2, :],
                             start=True, stop=True)
            nc.vector.tensor_copy(gstat[t][:, 0:2], gb_ps[:, 0:2])
            # mean/var/rstd -> per-channel affine a,b
            mean_t = left.tile([128, 1], f32, name=f"mean{t}", tag=f"mean{t}")
            e2_t = left.tile([128, 1], f32, name=f"e2{t}", tag=f"e2{t}")
            var_t = left.tile([128, 1], f32, name=f"var{t}", tag=f"var{t}")
            std_t = left.tile([128, 1], f32, name=f"std{t}", tag=f"std{t}")
            a_t = left.tile([128, 1], f32, name=f"a{t}", tag=f"a{t}")
            b_t = left.tile([128, 1], f32, name=f"b{t}", tag=f"b{t}")
            inv = 1.0 / GELEM
            nc.vector.tensor_scalar(mean_t[:], gstat[t][:, 0:1], inv, None, Alu.mult)
            nc.vector.tensor_scalar(e2_t[:], gstat[t][:, 1:2], inv, None, Alu.mult)
            nc.vector.scalar_tensor_tensor(
                var_t[:], mean_t[:], -1.0, mean_t[:], Alu.mult, Alu.mult)
            nc.vector.scalar_tensor_tensor(
                var_t[:], e2_t[:], EPS, var_t[:], Alu.add, Alu.add)
            nc.scalar.activation(std_t[:], var_t[:], Act.Sqrt)
            nc.vector.reciprocal(a_t[:], std_t[:])
            nc.vector.tensor_tensor(a_t[:], a_t[:], nwt[t][:], Alu.mult)
            nc.vector.tensor_tensor(b_t[:], mean_t[:], a_t[:], Alu.mult)
            nc.vector.tensor_tensor(b_t[:], nbt[t][:], b_t[:], Alu.subtract)
            # phase 2: normalize + cast
            nc.vector.tensor_scalar(
                xn[t][:], xf[t][:], a_t[:, 0:1], b_t[:, 0:1], Alu.mult, Alu.add)

        right_ctx.close()

        # ---------------- mid pools (reuse xf space) ----------------
        mid = ctx.enter_context(tc.tile_pool(name="mid", bufs=1))
        psb_pool = ctx.enter_context(tc.tile_pool(name="psb_pool", bufs=4))
        rec_pool = ctx.enter_context(tc.tile_pool(name="rec_pool", bufs=2))
        yo_pool = ctx.enter_context(tc.tile_pool(name="yo_pool", bufs=2))

        vT = mid.tile([128, MT * 520], bf16, name="vT", tag="vT")
        yh = [mid.tile([128, 512], f32, name=f"yh{i}", tag=f"yh{i}") for i in range(T)]
        attn = [mid.tile([128, NQ], bf16, name=f"attn{t}", tag=f"attn{t}") for t in range(T)]
        xres = [mid.tile([128, NQ], f32, name=f"xres{t}", tag=f"xres{t}") for t in range(T)]
        for t in range(T):
            nc.sync.dma_start(xres[t][:], xb[128 * t:128 * (t + 1), 0:NQ])

        # ones columns of the augmented v^T (denominator trick)
        ones_view = vT[:].rearrange("p (m h x) -> p m h x", m=MT, x=65)[:, :, :, 64:65]
        nc.vector.memset(ones_view, 1.0)

        # ---------------- phase 3: projections ----------------
        # q: out rows 0..511 of qkv, only window-0 columns here; window-1 q
        # and k tiles 1..3 are emitted later as attention-period fillers.
        def q_chain(i, w, par):
            qp = pool_ab[par].tile([128, 512], f32, name="qp",
                                   tag="sA" if par == 0 else "sB")
            for k in range(T):
                nc.tensor.matmul(
                    qp[:], wTq[k][:, 128 * i:128 * i + 128],
                    xn[k][:, 512 * w:512 * w + 512],
                    start=(k == 0), stop=(k == T - 1))
            nc.vector.tensor_scalar(
                qsb[i][:, 512 * w:512 * w + 512], qp[:], qb[i][:, 0:1], None, Alu.add)

        def k_chain(i, w, par):
            kp = pool_ab[par].tile([128, 512], f32, name="kp",
                                   tag="sA" if par == 0 else "sB")
            for k in range(T):
                nc.tensor.matmul(
                    kp[:], wTq[k][:, 512 + 128 * i:512 + 128 * i + 128],
                    xn[k][:, 512 * w:512 * w + 512],
                    start=(k == 0), stop=(k == T - 1))
            nc.vector.tensor_scalar(
                ksb[i][:, 512 * w:512 * w + 512], kp[:], qb[4 + i][:, 0:1], None, Alu.add)

        def v_chain(mt, par):
            vp = pool_ab[par].tile([128, 512], f32, name="vp",
                                   tag="sA" if par == 0 else "sB")
            for k in range(T):
                nc.tensor.matmul(
                    vp[:], xn[k][:, 128 * mt:128 * mt + 128],
                    wTq[k][:, 1024:1536],
                    start=(k == 0), stop=(k == T - 1))
            dst = vT[:, 520 * mt:520 * mt + 520].rearrange(
                "p (h x) -> p h x", x=65)[:, :, 0:64]
            srcv = vp[:].rearrange("p (h x) -> p h x", x=64)
            vbv = vb_bc[:].rearrange("p (h x) -> p h x", x=64)
            nc.vector.tensor_tensor(dst, srcv, vbv, Alu.add)

        def proj_chain(i, w, par, ks=(0, 1, 2, 3), partial=None, combine=None):
            py = pool_ab[par].tile([128, 512], f32, name="py",
                                   tag="sA" if par == 0 else "sB")
            for n_, k in enumerate(ks):
                nc.tensor.matmul(
                    py[:], wTp[k][:, 128 * i:128 * i + 128],
                    attn[k][:, 512 * w:512 * w + 512],
                    start=(n_ == 0), stop=(n_ == len(ks) - 1))
            if partial is not None:
                nc.vector.tensor_copy(partial[:], py[:])
                return
            yo = yo_pool.tile([128, 512], f32, name="yo", tag="yo")
            nc.vector.scalar_tensor_tensor(
                yo[:], py[:], pb[i][:, 0:1], xres[i][:, 512 * w:512 * w + 512],
                Alu.add, Alu.add)
            if combine is not None:
                nc.vector.tensor_tensor(yo[:], yo[:], combine[:], Alu.add)
            nc.sync.dma_start(y[128 * i:128 * i + 128, 512 * w:512 * w + 512], yo[:])

        # prefix: k tiles 0-1, window-0 q, all of vT (PV consumes vT from
        # the first attention period on)
        for w8 in range(8):
            k_chain(0, w8, w8 % 2)
            k_chain(1, w8, (w8 + 1) % 2)
        for i in range(T):
            q_chain(i, 0, i % 2)
        for mt in range(MT):
            v_chain(mt, mt % 2)

        # ---------------- phase 4: attention (flat pipelined stream) ------
        # Global stream of periods over (window, pair, group). PV runs one
        # period behind S/exp; pair normalize is deferred into the next
        # pair's first period; filler chains (k tiles 1-3, window-1 q,
        # window-0 proj) are emitted on alternate periods.
        def gsize(r):
            return 3 if r < NGRP - 1 else MT - 3 * (NGRP - 1)

        periods = [(w, p, r) for w in range(W) for p in range(4)
                   for r in range(NGRP)]
        pair_state = {}

        def emit_pv(w, p, r):
            pvs, ps_t = pair_state[(w, p)]
            if pvs[0] is None:
                for hh in range(2):
                    pvs[hh] = psum_pv.tile([128, 512], f32, name=f"pv{hh}", tag="pv")
            gs = gsize(r)
            for hh in range(2):
                h = 2 * p + hh
                pst = ps_t[hh][r]
                for j in range(gs):
                    m = 3 * r + j
                    nc.tensor.matmul(
                        pvs[hh][0:65, :],
                        vT[:, 520 * m + 65 * h:520 * m + 65 * h + 65],
                        pst[:, 512 * j:512 * j + 512],
                        start=(m == 0), stop=(m == MT - 1))

        def emit_normalize(w, p):
            pvs, _ = pair_state[(w, p)]
            for hh in range(2):
                h = 2 * p + hh
                kt, prr = h // 2, 64 * (h % 2)
                dnm = rec_pool.tile([1, 512], f32, name="dnm", tag="dnm")
                nc.vector.tensor_copy(dnm[0:1, :], pvs[hh][64:65, :])
                rec = rec_pool.tile([1, 512], f32, name="rec", tag="rec")
                rscr = rec_pool.tile([1, 512], f32, name="rscr", tag="rscr")
                nc.vector.reciprocal_approx_accurate(
                    rec[0:1, :], dnm[0:1, :], rscr[0:1, :])
                bc = pool_ab[hh].tile([64, 512], f32, name="bc",
                                     tag="sA" if hh == 0 else "sB")
                nc.tensor.matmul(
                    bc[0:64, :], ones_row[0:1, 0:D],
                    rec[0:1, :], start=True, stop=True)
                bcs = rec_pool.tile([64, 512], f32, name="bcs", tag="bcs")
                nc.vector.tensor_copy(bcs[0:64, :], bc[0:64, :])
                nc.vector.tensor_tensor(
                    attn[kt][prr:prr + 64, 512 * w:512 * w + 512],
                    pvs[hh][0:64, :], bcs[0:64, :], Alu.mult)

        # filler schedule: (earliest_period, closure); one pop on EVEN
        # periods, always from pool B (its next S alloc has ~2x more slack
        # than pool A's, so the filler's drain never delays the exp stream).
        # Emission deadlines: ksb[2] before period 22, ksb[3] before 33,
        # window-1 q before 44, window-0 proj after normalize(w0,p3) at 44.
        fillers = []
        for i in range(2, T):                       # ksb[2..3]: pops 2..16, 18..32
            for w8 in range(8):
                fillers.append((16 * (i - 2) + 2 + 2 * w8,
                                lambda i=i, w8=w8: k_chain(i, w8, 1)))
        for i in range(T):                          # window-1 q: pops 34..40
            fillers.append((34 + 2 * i, lambda i=i: q_chain(i, 1, 1)))
        for i in range(T):                          # window-0 proj: pops 46..52
            fillers.append((46 + 2 * i, lambda i=i: proj_chain(i, 0, 1)))
        for i in range(T):                          # w1 proj half (pairs 0-1)
            fillers.append((70 + 2 * i, lambda i=i:
                            proj_chain(i, 1, 1, ks=(0, 1), partial=yh[i])))
        fillers.reverse()   # pop from the end

        for g, (w, p, r) in enumerate(periods):
            gs = gsize(r)
            if r == 0:
                pair_state[(w, p)] = (
                    [None, None], [[None] * NGRP, [None] * NGRP])
            pvs, ps_t = pair_state[(w, p)]
            for hh in range(2):
                pr = 64 * hh
                sp = pool_ab[hh].tile([128, 512 * gs], f32, name=f"sp{hh}",
                                      tag="sA" if hh == 0 else "sB")
                for j in range(gs):
                    m = 3 * r + j
                    nc.tensor.matmul(
                        sp[:, 512 * j:512 * j + 512],
                        ksb[p][pr:pr + 64, 128 * m:128 * m + 128],
                        qsb[p][pr:pr + 64, 512 * w:512 * w + 512],
                        start=True, stop=True)
                pst = psb_pool.tile([128, 1536], bf16, name="ps", tag="ps")
                nc.scalar.activation(
                    pst[:, 0:512 * gs], sp[:, 0:512 * gs], Act.Exp, scale=0.125)
                ps_t[hh][r] = pst
                # PV of the previous period goes between the two S blocks
                # (h0 part) and after them (h1 part)
                if hh == 0:
                    if g > 0:
                        pw, pp, prr_ = periods[g - 1]
                        emit_pv(pw, pp, prr_)
            if g > 0 and periods[g - 1][2] == NGRP - 1:
                emit_normalize(*periods[g - 1][:2])
            if g % 2 == 0 and fillers and fillers[-1][0] <= g:
                fillers.pop()[1]()

        # tail: last period's PV, last normalize, window-1 proj second half
        emit_pv(*periods[-1])
        emit_normalize(W - 1, 3)
        while fillers:
            fillers.pop()[1]()
        for i in range(T):
            proj_chain(i, 1, i % 2, ks=(2, 3), combine=yh[i])


def _build():
    import concourse.tile as tile
    from concourse import bacc, mybir

    nc = bacc.Bacc("TRN2", target_bir_lowering=False, debug=False)
    f32 = mybir.dt.float32
    io = {
        "xb": nc.dram_tensor("xb", [C, N], f32, kind="ExternalInput").ap(),
        "qkvw": nc.dram_tensor("qkvw", [3 * C, C], f32, kind="ExternalInput").ap(),
        "qkvb": nc.dram_tensor("qkvb", [3 * C], f32, kind="ExternalInput").ap(),
        "projw": nc.dram_tensor("projw", [C, C], f32, kind="ExternalInput").ap(),
        "projb": nc.dram_tensor("projb", [C], f32, kind="ExternalInput").ap(),
        "nw": nc.dram_tensor("nw", [C], f32, kind="ExternalInput").ap(),
        "nb": nc.dram_tensor("nb", [C], f32, kind="ExternalInput").ap(),
        "cid": nc.dram_tensor("cid", [128, 128], mybir.dt.bfloat16,
                              kind="ExternalInput").ap(),
        "cind": nc.dram_tensor("cind", [128, 2], f32, kind="ExternalInput").ap(),
        "cindT": nc.dram_tensor("cindT", [2, 128], f32, kind="ExternalInput").ap(),
        "y": nc.dram_tensor("y", [C, NQ], f32, kind="ExternalOutput").ap(),
    }
    with tile.TileContext(nc) as tc:
        _emit(tc, io)
    nc.compile()
    return nc


def get_compiled():
    global _COMPILED
    if _COMPILED is None:
        _COMPILED = _build()
    return _COMPILED


def make_in_maps(x, norm_w, norm_b, qkv_w, qkv_b, proj_w, proj_b):
    import ml_dtypes

    xf = np.ascontiguousarray(np.asarray(x, np.float32)).reshape(2, C, N)
    ind = np.zeros((128, 2), np.float32)
    ind[0:64, 0] = 1.0
    ind[64:128, 1] = 1.0
    shared = {
        "cid": np.eye(128, dtype=ml_dtypes.bfloat16),
        "cind": ind,
        "cindT": np.ascontiguousarray(ind.T),
        "qkvw": np.ascontiguousarray(np.asarray(qkv_w, np.float32)),
        "qkvb": np.ascontiguousarray(np.asarray(qkv_b, np.float32)),
        "projw": np.ascontiguousarray(np.asarray(proj_w, np.float32)),
        "projb": np.ascontiguousarray(np.asarray(proj_b, np.float32)),
        "nw": np.ascontiguousarray(np.asarray(norm_w, np.float32)),
        "nb": np.ascontiguousarray(np.asarray(norm_b, np.float32)),
    }
    in_maps = []
    for core in range(8):
        bi, qs = core // 4, core % 4
        # rotate so this core's queries are always columns [0:NQ)
        xroll = np.concatenate(
            [xf[bi][:, qs * NQ:], xf[bi][:, :qs * NQ]], axis=1)
        m = dict(shared)
        m["xb"] = np.ascontiguousarray(xroll)
        in_maps.append(m)
    return in_maps


def assemble(results, x):
    y = np.zeros((2, C, N), np.float32)
    for core in range(8):
        bi, qs = core // 4, core % 4
        y[bi][:, qs * NQ:(qs + 1) * NQ] = results[core]["y"]
    return y.reshape(x.shape)


def kernel(x, norm_w, norm_b, qkv_w, qkv_b, proj_w, proj_b, **_ignored):
    from concourse import bass_utils

    nc = get_compiled()
    in_maps = make_in_maps(x, norm_w, norm_b, qkv_w, qkv_b, proj_w, proj_b)
    res = bass_utils.run_bass_kernel_spmd(nc, in_maps, core_ids=list(range(8)))
    return assemble(res.results, np.asarray(x))
